# revision 7
# baseline (speedup 1.0000x reference)
"""Multi-head self-attention (B=4, T=2048, E=1024, H=16) on 8 trn2 NeuronCores.

Sharding: core (b, h) = batch b, token-half h. Each core computes K/V for the
full sequence (duplicated within the batch pair), Q for its own 8 query blocks
of 128 tokens, causal attention for those blocks, then the output projection
and LayerNorm for its own tokens. Causal balance: query blocks are paired
(j, 15-j) so both cores process blocks with padded key-lengths 2,4,...,16;
host-supplied mask tiles encode the true causal structure, keeping the
compiled program identical across cores (SPMD).

All matmuls run in bf16 with fp32 PSUM accumulation (validated ~2e-3
scale-relative error vs the fp32 reference).
"""
import json
import numpy as np
import ml_dtypes
from contextlib import ExitStack

import concourse.bass as bass
import concourse.bass_utils as _bass_utils
import concourse.tile as tile
from concourse import mybir
from concourse.bass_utils import run_bass_kernel_spmd

# ----------------------------------------------------------------------------
# Toolchain workarounds for this container's walrus build (see birfix notes):
# 1. EVENT_SEMAPHORE_RANGE_CLEAR InstISA is rejected ("ISA wrong length").
# 2. Engine instructions only carry one semaphore-wait slot; extra waits are
#    peeled onto NoOp carriers on the same engine (order-preserving).
# ----------------------------------------------------------------------------


def _patched_clear_and_free_semaphores(self, sems):
    if not sems:
        return
    sem_nums = [s.num if hasattr(s, "num") else s for s in sems]
    self._state.prepend_free_semaphores(sem_nums)
    for poison_set in self._tile_sem_poison_stack:
        poison_set.update(sem_nums)


def _fix_bir_waits(bir_json: bytes) -> bytes:
    bir = json.loads(bir_json)
    ctr = 0
    changed = False
    for func in bir.get("functions", []):
        for blk in func.get("blocks", []):
            out = []
            for inst in blk.get("instructions", []):
                si = inst.get("sync_info") or {}
                waits = si.get("on_wait") or []
                if len(waits) > 1:
                    for w in waits[:-1]:
                        ctr += 1
                        out.append(
                            {
                                "debug": inst.get("debug"),
                                "engine": inst.get("engine", "SP"),
                                "ins": [],
                                "name": f"IWF-{ctr}",
                                "opcode": "NoOp",
                                "outs": [],
                                "sync_info": {"on_wait": [w]},
                            }
                        )
                    si = dict(si)
                    si["on_wait"] = waits[-1:]
                    inst = dict(inst)
                    inst["sync_info"] = si
                    changed = True
                out.append(inst)
            blk["instructions"] = out
    return json.dumps(bir).encode() if changed else bir_json


_orig_compile_bir_kernel = _bass_utils.compile_bir_kernel


def _patched_compile_bir_kernel(bir_json, tmpdir, neff_name="file.neff"):
    if isinstance(bir_json, str):
        bir_json = bir_json.encode()
    return _orig_compile_bir_kernel(_fix_bir_waits(bir_json), tmpdir, neff_name)


def _install_patches():
    if getattr(bass.Bass, "_mhsa_patched", False):
        return
    bass.Bass.clear_and_free_semaphores = _patched_clear_and_free_semaphores
    bass.Bass._mhsa_patched = True
    _bass_utils.compile_bir_kernel = _patched_compile_bir_kernel
    try:
        import concourse.bass2jax as _b2j

        _b2j.compile_bir_kernel = _patched_compile_bir_kernel
    except ImportError:
        pass


_install_patches()

# ----------------------------------------------------------------------------
# Problem constants (hardcoded per spec)
# ----------------------------------------------------------------------------
B, T, E, H = 4, 2048, 1024, 16
HD = E // H  # 64
P = 128
NB = T // P  # 16 query/key blocks
NQ = 8  # query blocks per core
EC = E // P  # 8 e-chunks
SCALE = 1.0 / float(np.sqrt(T))
EPS = 1e-6
BF = mybir.dt.bfloat16
F32 = mybir.dt.float32
NPBF = ml_dtypes.bfloat16

# query-block assignment: pairs (j, 15-j); core h=0 takes even-j pairs' low
# and high ends so both cores see padded lengths L_k = 2(k+1)
BLOCKS_A = [0, 2, 4, 6, 9, 11, 13, 15]  # true lengths 1,3,5,7,10,12,14,16
BLOCKS_B = [1, 3, 5, 7, 8, 10, 12, 14]  # true lengths 2,4,6,8,9,11,13,15
PAD_L = [2 * (k + 1) for k in range(NQ)]  # 2,4,...,16

_nc_cache = {}


def _build_nc():
    if "nc" in _nc_cache:
        return _nc_cache["nc"]
    nc = bass.Bass(num_devices=8)

    # inputs (per-core)
    xT_d = nc.dram_tensor("xT", [E, T], BF, kind="ExternalInput")
    xTq_d = nc.dram_tensor("xTq", [E, NQ * P], BF, kind="ExternalInput")
    WqT_d = nc.dram_tensor("WqT", [E, E], BF, kind="ExternalInput")
    WkT_d = nc.dram_tensor("WkT", [E, E], BF, kind="ExternalInput")
    WvT_d = nc.dram_tensor("WvT", [E, E], BF, kind="ExternalInput")
    WpT_d = nc.dram_tensor("WpT", [E, E], BF, kind="ExternalInput")
    bqT_d = nc.dram_tensor("bqT", [P, EC], F32, kind="ExternalInput")
    bkT_d = nc.dram_tensor("bkT", [P, EC], F32, kind="ExternalInput")
    bv_d = nc.dram_tensor("bv_bc", [P, E], F32, kind="ExternalInput")
    bp_d = nc.dram_tensor("bp_bc", [P, E], F32, kind="ExternalInput")
    gm_d = nc.dram_tensor("gamma_bc", [P, E], F32, kind="ExternalInput")
    bt_d = nc.dram_tensor("beta_bc", [P, E], F32, kind="ExternalInput")
    m1_d = nc.dram_tensor("m1", [P, NQ, P], BF, kind="ExternalInput")
    m2_d = nc.dram_tensor("m2", [P, NQ, P], BF, kind="ExternalInput")
    y_d = nc.dram_tensor("y", [NQ, P, E], F32, kind="ExternalOutput")

    with tile.TileContext(nc) as tc:
        with ExitStack() as ctx:
            consts = ctx.enter_context(tc.tile_pool(name="consts", bufs=1))
            big = ctx.enter_context(tc.tile_pool(name="big", bufs=1))
            wpool = ctx.enter_context(tc.tile_pool(name="wpool", bufs=1))
            work = ctx.enter_context(tc.tile_pool(name="work", bufs=2))
            ps = ctx.enter_context(tc.tile_pool(name="ps", bufs=1, space="PSUM"))

            # ---------------- constants / inputs to SBUF ----------------
            xT = big.tile([P, EC, T], BF)
            for c in range(EC):
                nc.sync.dma_start(
                    xT[:, c, :], xT_d.rearrange("(c p) t -> p c t", p=P)[:, c, :]
                )
            xTq = big.tile([P, EC, NQ * P], BF)
            for c in range(EC):
                nc.sync.dma_start(
                    xTq[:, c, :], xTq_d.rearrange("(c p) t -> p c t", p=P)[:, c, :]
                )
            bqT = consts.tile([P, EC], F32)
            nc.sync.dma_start(bqT[:], bqT_d[:, :])
            bkT = consts.tile([P, EC], F32)
            nc.sync.dma_start(bkT[:], bkT_d[:, :])
            bv_bc = consts.tile([P, E], F32)
            nc.sync.dma_start(bv_bc[:], bv_d[:, :])
            bp_bc = consts.tile([P, E], F32)
            nc.sync.dma_start(bp_bc[:], bp_d[:, :])
            gamma_bc = consts.tile([P, E], F32)
            nc.sync.dma_start(gamma_bc[:], gm_d[:, :])
            beta_bc = consts.tile([P, E], F32)
            nc.sync.dma_start(beta_bc[:], bt_d[:, :])
            m1 = consts.tile([P, NQ, P], BF)
            nc.sync.dma_start(m1[:], m1_d[:, :, :])
            m2 = consts.tile([P, NQ, P], BF)
            nc.sync.dma_start(m2[:], m2_d[:, :, :])
            ones64 = consts.tile([P, 64], F32)
            nc.vector.memset(ones64[:], 1.0)

            # persistent intermediates
            KT = big.tile([P, EC, T], BF)  # K^T  [f, t]
            QT = big.tile([P, EC, NQ * P], BF)  # Q^T  [f, t_own]
            Vx = big.tile([P, NB, H, HD + 1], BF)  # V ext [t, h, d|1]
            zT = big.tile([P, EC, NQ * P], BF)  # z^T  [e, t_own]
            nc.vector.memset(Vx[:, :, :, HD : HD + 1], 1.0)

            def load_w(dram, name):
                w = wpool.tile([P, EC, E], BF, tag="w", name=name)
                for c in range(EC):
                    nc.sync.dma_start(
                        w[:, c, :], dram.rearrange("(c p) f -> p c f", p=P)[:, c, :]
                    )
                return w

            # ---------------- K^T = Wk^T.T-chunks x xT + bk ----------------
            Wk = load_w(WkT_d, "Wk")
            for fb in range(EC):
                for ts_ in range(T // 512):
                    pk = ps.tile([P, 512], F32, tag="mm512", bufs=2, name="pk")
                    for c in range(EC):
                        nc.tensor.matmul(
                            pk[:],
                            Wk[:, c, fb * P : (fb + 1) * P],
                            xT[:, c, ts_ * 512 : (ts_ + 1) * 512],
                            start=(c == 0),
                            stop=(c == EC - 1),
                        )
                    nc.vector.tensor_scalar(
                        out=KT[:, fb, ts_ * 512 : (ts_ + 1) * 512],
                        in0=pk[:],
                        scalar1=bkT[:, fb : fb + 1],
                        scalar2=None,
                        op0=mybir.AluOpType.add,
                    )

            # ---------------- V = xT-chunks x Wv^T + bv (t-major, ext) -----
            Wv = load_w(WvT_d, "Wv")
            for tb in range(NB):
                for fs in range(E // 512):
                    pv = ps.tile([P, 512], F32, tag="mm512", bufs=2, name="pv")
                    for c in range(EC):
                        nc.tensor.matmul(
                            pv[:],
                            xT[:, c, tb * P : (tb + 1) * P],
                            Wv[:, c, fs * 512 : (fs + 1) * 512],
                            start=(c == 0),
                            stop=(c == EC - 1),
                        )
                    for hh in range(8):
                        h = fs * 8 + hh
                        nc.vector.tensor_tensor(
                            out=Vx[:, tb, h, 0:HD],
                            in0=pv[:, hh * HD : (hh + 1) * HD],
                            in1=bv_bc[:, h * HD : (h + 1) * HD],
                            op=mybir.AluOpType.add,
                        )

            # ---------------- Q^T = Wq^T-chunks x xTq + bq -----------------
            Wq = load_w(WqT_d, "Wq")
            for fb in range(EC):
                for ts_ in range(NQ * P // 512):
                    pq = ps.tile([P, 512], F32, tag="mm512", bufs=2, name="pq")
                    for c in range(EC):
                        nc.tensor.matmul(
                            pq[:],
                            Wq[:, c, fb * P : (fb + 1) * P],
                            xTq[:, c, ts_ * 512 : (ts_ + 1) * 512],
                            start=(c == 0),
                            stop=(c == EC - 1),
                        )
                    nc.vector.tensor_scalar(
                        out=QT[:, fb, ts_ * 512 : (ts_ + 1) * 512],
                        in0=pq[:],
                        scalar1=bqT[:, fb : fb + 1],
                        scalar2=None,
                        op0=mybir.AluOpType.add,
                    )

            # ---------------- attention ----------------
            for k_idx in range(NQ):
                L = PAD_L[k_idx]
                qs = slice(k_idx * P, (k_idx + 1) * P)
                for pr in range(H // 2):  # head pair = f-chunk pr
                    h_e, h_o = 2 * pr, 2 * pr + 1
                    pOe = ps.tile([P, P], F32, tag="Oe", bufs=1, name="pOe")
                    pOo = ps.tile([P, P], F32, tag="Oo", bufs=1, name="pOo")

                    def score_pair(j):
                        pSe = ps.tile([P, P], F32, tag="S", bufs=4, name="pSe")
                        pSo = ps.tile([P, P], F32, tag="S", bufs=4, name="pSo")
                        js = slice(j * P, (j + 1) * P)
                        nc.tensor.matmul(
                            pSe[:],
                            KT[0:64, pr, js],
                            QT[0:64, pr, qs],
                            start=True,
                            stop=True,
                            tile_position=(0, 0),
                        )
                        nc.tensor.matmul(
                            pSo[:],
                            KT[64:128, pr, js],
                            QT[64:128, pr, qs],
                            start=True,
                            stop=True,
                            tile_position=(64, 0),
                        )
                        return pSe, pSo

                    for j in range(L):
                        pSe, pSo = score_pair(j)
                        eSe = work.tile([P, P], BF, tag="eS", bufs=4, name="eSe")
                        eSo = work.tile([P, P], BF, tag="eS", bufs=4, name="eSo")
                        nc.scalar.activation(
                            eSe[:], pSe[:], mybir.ActivationFunctionType.Exp,
                            scale=SCALE,
                        )
                        nc.scalar.activation(
                            eSo[:], pSo[:], mybir.ActivationFunctionType.Exp,
                            scale=SCALE,
                        )
                        if j == L - 2:
                            nc.vector.tensor_tensor(
                                out=eSe[:], in0=eSe[:], in1=m1[:, k_idx, :],
                                op=mybir.AluOpType.mult,
                            )
                            nc.vector.tensor_tensor(
                                out=eSo[:], in0=eSo[:], in1=m1[:, k_idx, :],
                                op=mybir.AluOpType.mult,
                            )
                        elif j == L - 1:
                            nc.vector.tensor_tensor(
                                out=eSe[:], in0=eSe[:], in1=m2[:, k_idx, :],
                                op=mybir.AluOpType.mult,
                            )
                            nc.vector.tensor_tensor(
                                out=eSo[:], in0=eSo[:], in1=m2[:, k_idx, :],
                                op=mybir.AluOpType.mult,
                            )
                        nc.tensor.matmul(
                            pOe[0 : HD + 1, :],
                            Vx[:, j, h_e, :],
                            eSe[:],
                            start=(j == 0),
                            stop=(j == L - 1),
                        )
                        nc.tensor.matmul(
                            pOo[0 : HD + 1, :],
                            Vx[:, j, h_o, :],
                            eSo[:],
                            start=(j == 0),
                            stop=(j == L - 1),
                        )
                    # normalize + scatter into z^T
                    for h, pO in ((h_e, pOe), (h_o, pOo)):
                        rc = work.tile([P, P], F32, tag="rc", bufs=2, name="rc")
                        nc.vector.reciprocal(rc[64:65, :], pO[64:65, :])
                        pR = ps.tile([64, P], F32, tag="S", bufs=4, name="pR")
                        nc.tensor.matmul(
                            pR[:], ones64[64:65, :], rc[64:65, :],
                            start=True, stop=True,
                        )
                        Rs = work.tile([64, P], F32, tag="Rs", bufs=2, name="Rs")
                        nc.scalar.activation(
                            Rs[:], pR[:], mybir.ActivationFunctionType.Copy
                        )
                        zh = work.tile([64, P], BF, tag="zh", bufs=4, name="zh")
                        nc.vector.tensor_tensor(
                            out=zh[:], in0=pO[0:HD, :], in1=Rs[:],
                            op=mybir.AluOpType.mult,
                        )
                        nc.sync.dma_start(
                            zT[(h % 2) * 64 : (h % 2) * 64 + 64, h // 2, qs], zh[:]
                        )

            # residual: z^T += xTq
            for c in range(EC):
                nc.vector.tensor_tensor(
                    out=zT[:, c, :], in0=zT[:, c, :], in1=xTq[:, c, :],
                    op=mybir.AluOpType.add,
                )

            # ---------------- projection + layernorm ----------------
            Wp = load_w(WpT_d, "Wp")
            inv_e = 1.0 / float(E)
            for tb in range(NQ):
                y_sb = work.tile([P, E], F32, tag="ysb", bufs=2, name="y_sb")
                for fs in range(E // 512):
                    py = ps.tile([P, 512], F32, tag="mm512", bufs=2, name="py")
                    for c in range(EC):
                        nc.tensor.matmul(
                            py[:],
                            zT[:, c, tb * P : (tb + 1) * P],
                            Wp[:, c, fs * 512 : (fs + 1) * 512],
                            start=(c == 0),
                            stop=(c == EC - 1),
                        )
                    nc.vector.tensor_tensor(
                        out=y_sb[:, fs * 512 : (fs + 1) * 512],
                        in0=py[:],
                        in1=bp_bc[:, fs * 512 : (fs + 1) * 512],
                        op=mybir.AluOpType.add,
                    )
                mean = work.tile([P, 1], F32, tag="stat", bufs=8, name="mean")
                nc.vector.reduce_sum(mean[:], y_sb[:], axis=mybir.AxisListType.X)
                nc.vector.tensor_scalar_mul(mean[:], mean[:], inv_e)
                y_c = work.tile([P, E], F32, tag="yc", bufs=2, name="y_c")
                nc.vector.tensor_scalar(
                    out=y_c[:], in0=y_sb[:], scalar1=mean[:, 0:1], scalar2=None,
                    op0=mybir.AluOpType.subtract,
                )
                var = work.tile([P, 1], F32, tag="stat", bufs=8, name="var")
                nc.scalar.activation(
                    y_sb[:], y_c[:], mybir.ActivationFunctionType.Square,
                    accum_out=var[:],
                )
                rstd = work.tile([P, 1], F32, tag="stat", bufs=8, name="rstd")
                nc.vector.tensor_scalar(
                    out=rstd[:], in0=var[:], scalar1=inv_e, scalar2=float(EPS),
                    op0=mybir.AluOpType.mult, op1=mybir.AluOpType.add,
                )
                nc.scalar.activation(
                    rstd[:], rstd[:], mybir.ActivationFunctionType.Sqrt
                )
                nc.vector.reciprocal(rstd[:], rstd[:])
                nc.vector.tensor_scalar_mul(y_c[:], y_c[:], rstd[:, 0:1])
                nc.vector.tensor_tensor(
                    out=y_c[:], in0=y_c[:], in1=gamma_bc[:],
                    op=mybir.AluOpType.mult,
                )
                nc.vector.tensor_tensor(
                    out=y_c[:], in0=y_c[:], in1=beta_bc[:],
                    op=mybir.AluOpType.add,
                )
                nc.sync.dma_start(y_d[tb, :, :], y_c[:])

    _nc_cache["nc"] = nc
    return nc


def _make_masks(blocks):
    m1 = np.zeros((NQ, P, P), np.float32)
    m2 = np.zeros((NQ, P, P), np.float32)
    tril_t = (np.arange(P)[:, None] <= np.arange(P)[None, :]).astype(np.float32)
    for k in range(NQ):
        l_true = blocks[k] + 1
        L = PAD_L[k]
        if l_true == L:
            m1[k] = 1.0
            m2[k] = tril_t
        else:
            assert l_true == L - 1
            m1[k] = tril_t
            m2[k] = 0.0
    # device layout [P(k-local), NQ, P(q-local)]
    return (
        np.ascontiguousarray(m1.transpose(1, 0, 2)).astype(NPBF),
        np.ascontiguousarray(m2.transpose(1, 0, 2)).astype(NPBF),
    )


def kernel(x, Wq, bq, Wk, bk, Wv, bv, Wp, bp, gamma, beta):
    x = np.asarray(x, np.float32)
    nc = _build_nc()

    WqT = np.ascontiguousarray(np.asarray(Wq, np.float32).T).astype(NPBF)
    WkT = np.ascontiguousarray(np.asarray(Wk, np.float32).T).astype(NPBF)
    WvT = np.ascontiguousarray(np.asarray(Wv, np.float32).T).astype(NPBF)
    WpT = np.ascontiguousarray(np.asarray(Wp, np.float32).T).astype(NPBF)
    bqT = np.ascontiguousarray(np.asarray(bq, np.float32).reshape(EC, P).T)
    bkT = np.ascontiguousarray(np.asarray(bk, np.float32).reshape(EC, P).T)
    bv_bc = np.ascontiguousarray(
        np.broadcast_to(np.asarray(bv, np.float32), (P, E))
    )
    bp_bc = np.ascontiguousarray(
        np.broadcast_to(np.asarray(bp, np.float32), (P, E))
    )
    gamma_bc = np.ascontiguousarray(
        np.broadcast_to(np.asarray(gamma, np.float32), (P, E))
    )
    beta_bc = np.ascontiguousarray(
        np.broadcast_to(np.asarray(beta, np.float32), (P, E))
    )
    masks = {0: _make_masks(BLOCKS_A), 1: _make_masks(BLOCKS_B)}

    in_maps = []
    for core in range(8):
        b, h = core // 2, core % 2
        blocks = BLOCKS_A if h == 0 else BLOCKS_B
        own = np.concatenate([np.arange(blk * P, (blk + 1) * P) for blk in blocks])
        xb = x[b]  # (T, E)
        xT = np.ascontiguousarray(xb.T).astype(NPBF)
        xTq = np.ascontiguousarray(xb[own].T).astype(NPBF)
        m1c, m2c = masks[h]
        in_maps.append(
            {
                "xT": xT,
                "xTq": xTq,
                "WqT": WqT,
                "WkT": WkT,
                "WvT": WvT,
                "WpT": WpT,
                "bqT": bqT,
                "bkT": bkT,
                "bv_bc": bv_bc,
                "bp_bc": bp_bc,
                "gamma_bc": gamma_bc,
                "beta_bc": beta_bc,
                "m1": m1c,
                "m2": m2c,
            }
        )

    import os

    trace = bool(int(os.environ.get("MHSA_TRACE", "0")))
    res = run_bass_kernel_spmd(
        nc, in_maps, core_ids=list(range(8)), trace=trace,
        trace_cores=list(range(8)) if trace else None,
    )
    if trace and res.exec_time_ns is not None:
        print(f"HW exec time: {res.exec_time_ns} ns")
        if res.mean_exec_time_ns is not None:
            print(f"HW exec mean across cores: {res.mean_exec_time_ns:.0f} ns")
        kernel.last_exec_time_ns = res.exec_time_ns
        kernel.last_trace = res.instructions_and_trace

    out = np.empty((B, T, E), np.float32)
    for core in range(8):
        b, h = core // 2, core % 2
        blocks = BLOCKS_A if h == 0 else BLOCKS_B
        y = res.results[core]["y"]  # (NQ, P, E)
        for k, blk in enumerate(blocks):
            out[b, blk * P : (blk + 1) * P, :] = y[k]
    return out


# revision 16
# speedup vs baseline: 1.4194x; 1.4194x over previous
"""Multi-head self-attention (B=4, T=2048, E=1024, H=16) on 8 trn2 NeuronCores.

Sharding: core (b, h) = batch b, token-half h. Each core computes K/V for the
full sequence (duplicated within the batch pair), Q for its own 8 query blocks
of 128 tokens, causal attention for those blocks, then the output projection
and LayerNorm for its own tokens. Causal balance: query blocks are paired
(j, 15-j) so both cores process blocks with padded key-lengths 2,4,...,16;
host-supplied mask tiles encode the true causal structure, keeping the
compiled program identical across cores (SPMD).

All matmuls run in bf16 with fp32 PSUM accumulation (validated ~2e-3
scale-relative error vs the fp32 reference).
"""
import json
import numpy as np
import ml_dtypes
from contextlib import ExitStack

import concourse.bass as bass
import concourse.bass_utils as _bass_utils
import concourse.tile as tile
from concourse import mybir
from concourse.bass_utils import run_bass_kernel_spmd

# ----------------------------------------------------------------------------
# Toolchain workarounds for this container's walrus build (see birfix notes):
# 1. EVENT_SEMAPHORE_RANGE_CLEAR InstISA is rejected ("ISA wrong length").
# 2. Engine instructions only carry one semaphore-wait slot; extra waits are
#    peeled onto NoOp carriers on the same engine (order-preserving).
# ----------------------------------------------------------------------------


def _patched_clear_and_free_semaphores(self, sems):
    if not sems:
        return
    sem_nums = [s.num if hasattr(s, "num") else s for s in sems]
    self._state.prepend_free_semaphores(sem_nums)
    for poison_set in self._tile_sem_poison_stack:
        poison_set.update(sem_nums)


def _fix_bir_waits(bir_json: bytes) -> bytes:
    bir = json.loads(bir_json)
    ctr = 0
    changed = False
    for func in bir.get("functions", []):
        for blk in func.get("blocks", []):
            out = []
            for inst in blk.get("instructions", []):
                si = inst.get("sync_info") or {}
                waits = si.get("on_wait") or []
                if len(waits) > 1:
                    for w in waits[:-1]:
                        ctr += 1
                        out.append(
                            {
                                "debug": inst.get("debug"),
                                "engine": inst.get("engine", "SP"),
                                "ins": [],
                                "name": f"IWF-{ctr}",
                                "opcode": "NoOp",
                                "outs": [],
                                "sync_info": {"on_wait": [w]},
                            }
                        )
                    si = dict(si)
                    si["on_wait"] = waits[-1:]
                    inst = dict(inst)
                    inst["sync_info"] = si
                    changed = True
                out.append(inst)
            blk["instructions"] = out
    return json.dumps(bir).encode() if changed else bir_json


_orig_compile_bir_kernel = _bass_utils.compile_bir_kernel


def _patched_compile_bir_kernel(bir_json, tmpdir, neff_name="file.neff"):
    if isinstance(bir_json, str):
        bir_json = bir_json.encode()
    return _orig_compile_bir_kernel(_fix_bir_waits(bir_json), tmpdir, neff_name)


def _install_patches():
    if getattr(bass.Bass, "_mhsa_patched", False):
        return
    bass.Bass.clear_and_free_semaphores = _patched_clear_and_free_semaphores
    bass.Bass._mhsa_patched = True
    _bass_utils.compile_bir_kernel = _patched_compile_bir_kernel
    try:
        import concourse.bass2jax as _b2j

        _b2j.compile_bir_kernel = _patched_compile_bir_kernel
    except ImportError:
        pass


_install_patches()

# ----------------------------------------------------------------------------
# Problem constants (hardcoded per spec)
# ----------------------------------------------------------------------------
B, T, E, H = 4, 2048, 1024, 16
HD = E // H  # 64
P = 128
NB = T // P  # 16 query/key blocks
NQ = 8  # query blocks per core
EC = E // P  # 8 e-chunks
SCALE = 1.0 / float(np.sqrt(T))
EPS = 1e-6
BF = mybir.dt.bfloat16
F32 = mybir.dt.float32
NPBF = ml_dtypes.bfloat16

# query-block assignment: pairs (j, 15-j); core h=0 takes even-j pairs' low
# and high ends so both cores see padded lengths L_k = 2(k+1)
BLOCKS_A = [0, 2, 4, 6, 9, 11, 13, 15]  # true lengths 1,3,5,7,10,12,14,16
BLOCKS_B = [1, 3, 5, 7, 8, 10, 12, 14]  # true lengths 2,4,6,8,9,11,13,15
PAD_L = [2 * (k + 1) for k in range(NQ)]  # 2,4,...,16

_nc_cache = {}


def _build_nc():
    if "nc" in _nc_cache:
        return _nc_cache["nc"]
    nc = bass.Bass(num_devices=8)

    # inputs (per-core)
    xT_d = nc.dram_tensor("xT", [E, T], BF, kind="ExternalInput")
    xTq_d = nc.dram_tensor("xTq", [E, NQ * P], BF, kind="ExternalInput")
    WqT_d = nc.dram_tensor("WqT", [E, E], BF, kind="ExternalInput")
    WkT_d = nc.dram_tensor("WkT", [E, E], BF, kind="ExternalInput")
    WvT_d = nc.dram_tensor("WvT", [E, E], BF, kind="ExternalInput")
    WpT_d = nc.dram_tensor("WpT", [E, E], BF, kind="ExternalInput")
    bqT_d = nc.dram_tensor("bqT", [P, EC], F32, kind="ExternalInput")
    bkT_d = nc.dram_tensor("bkT", [P, EC], F32, kind="ExternalInput")
    bv_d = nc.dram_tensor("bv_bc", [P, E], F32, kind="ExternalInput")
    bp_d = nc.dram_tensor("bp_bc", [P, E], F32, kind="ExternalInput")
    gm_d = nc.dram_tensor("gamma_bc", [P, E], F32, kind="ExternalInput")
    bt_d = nc.dram_tensor("beta_bc", [P, E], F32, kind="ExternalInput")
    m1_d = nc.dram_tensor("m1", [P, NQ, P], BF, kind="ExternalInput")
    m2_d = nc.dram_tensor("m2", [P, NQ, P], BF, kind="ExternalInput")
    y_d = nc.dram_tensor("y", [NQ, P, E], F32, kind="ExternalOutput")

    with tile.TileContext(nc) as tc:
        with ExitStack() as ctx:
            consts = ctx.enter_context(tc.tile_pool(name="consts", bufs=1))
            big = ctx.enter_context(tc.tile_pool(name="big", bufs=1))
            wpool = ctx.enter_context(tc.tile_pool(name="wpool", bufs=1))
            work = ctx.enter_context(tc.tile_pool(name="work", bufs=2))
            # QKV-phase PSUM pool; closed before attention so its banks are
            # reused by the attention pool (8-bank budget)
            _psA_cm = tc.tile_pool(name="psA", bufs=1, space="PSUM")
            ps = _psA_cm.__enter__()

            # ---------------- constants / inputs to SBUF ----------------
            xT = big.tile([P, EC, T], BF)
            for c in range(EC):
                nc.sync.dma_start(
                    xT[:, c, :], xT_d.rearrange("(c p) t -> p c t", p=P)[:, c, :]
                )
            xTq = big.tile([P, EC, NQ * P], BF)
            for c in range(EC):
                nc.sync.dma_start(
                    xTq[:, c, :], xTq_d.rearrange("(c p) t -> p c t", p=P)[:, c, :]
                )
            bqT = consts.tile([P, EC], F32)
            nc.sync.dma_start(bqT[:], bqT_d[:, :])
            bkT = consts.tile([P, EC], F32)
            nc.sync.dma_start(bkT[:], bkT_d[:, :])
            bv_bc = consts.tile([P, E], F32)
            nc.sync.dma_start(bv_bc[:], bv_d[:, :])
            bp_bc = consts.tile([P, E], F32)
            nc.sync.dma_start(bp_bc[:], bp_d[:, :])
            gamma_bc = consts.tile([P, E], F32)
            nc.sync.dma_start(gamma_bc[:], gm_d[:, :])
            beta_bc = consts.tile([P, E], F32)
            nc.sync.dma_start(beta_bc[:], bt_d[:, :])
            m1 = consts.tile([P, NQ, P], BF)
            nc.sync.dma_start(m1[:], m1_d[:, :, :])
            m2 = consts.tile([P, NQ, P], BF)
            nc.sync.dma_start(m2[:], m2_d[:, :, :])
            ones64 = consts.tile([P, 64], F32)
            nc.vector.memset(ones64[:], 1.0)

            # persistent intermediates
            KT = big.tile([P, EC, T], BF)  # K^T  [f, t]
            QT = big.tile([P, EC, NQ * P], BF)  # Q^T  [f, t_own]
            Vx = big.tile([P, NB, H, HD + 1], BF)  # V ext [t, h, d|1]
            zT = big.tile([P, EC, NQ * P], BF)  # z^T  [e, t_own]
            nc.vector.memset(Vx[:, :, :, HD : HD + 1], 1.0)

            def load_w(dram, name):
                w = wpool.tile([P, EC, E], BF, tag="w", name=name)
                for c in range(EC):
                    nc.sync.dma_start(
                        w[:, c, :], dram.rearrange("(c p) f -> p c f", p=P)[:, c, :]
                    )
                return w

            # ---------------- K^T = Wk^T.T-chunks x xT + bk ----------------
            Wk = load_w(WkT_d, "Wk")
            for fb in range(EC):
                for ts_ in range(T // 512):
                    pk = ps.tile([P, 512], F32, tag="mm512", bufs=2, name="pk")
                    for c in range(EC):
                        nc.tensor.matmul(
                            pk[:],
                            Wk[:, c, fb * P : (fb + 1) * P],
                            xT[:, c, ts_ * 512 : (ts_ + 1) * 512],
                            start=(c == 0),
                            stop=(c == EC - 1),
                        )
                    nc.vector.tensor_scalar(
                        out=KT[:, fb, ts_ * 512 : (ts_ + 1) * 512],
                        in0=pk[:],
                        scalar1=bkT[:, fb : fb + 1],
                        scalar2=None,
                        op0=mybir.AluOpType.add,
                    )

            # ---------------- V = xT-chunks x Wv^T + bv (t-major, ext) -----
            Wv = load_w(WvT_d, "Wv")
            for tb in range(NB):
                for fs in range(E // 512):
                    pv = ps.tile([P, 512], F32, tag="mm512", bufs=2, name="pv")
                    for c in range(EC):
                        nc.tensor.matmul(
                            pv[:],
                            xT[:, c, tb * P : (tb + 1) * P],
                            Wv[:, c, fs * 512 : (fs + 1) * 512],
                            start=(c == 0),
                            stop=(c == EC - 1),
                        )
                    nc.vector.tensor_tensor(
                        out=Vx[:, tb, fs * 8 : (fs + 1) * 8, 0:HD],
                        in0=pv[:, :].rearrange("p (h d) -> p h d", d=HD),
                        in1=bv_bc[:, fs * 512 : (fs + 1) * 512].rearrange(
                            "p (h d) -> p h d", d=HD
                        ),
                        op=mybir.AluOpType.add,
                    )

            # ---------------- Q^T = Wq^T-chunks x xTq + bq -----------------
            Wq = load_w(WqT_d, "Wq")
            for fb in range(EC):
                for ts_ in range(NQ * P // 512):
                    pq = ps.tile([P, 512], F32, tag="mm512", bufs=2, name="pq")
                    for c in range(EC):
                        nc.tensor.matmul(
                            pq[:],
                            Wq[:, c, fb * P : (fb + 1) * P],
                            xTq[:, c, ts_ * 512 : (ts_ + 1) * 512],
                            start=(c == 0),
                            stop=(c == EC - 1),
                        )
                    nc.vector.tensor_scalar(
                        out=QT[:, fb, ts_ * 512 : (ts_ + 1) * 512],
                        in0=pq[:],
                        scalar1=bqT[:, fb : fb + 1],
                        scalar2=None,
                        op0=mybir.AluOpType.add,
                    )

            # ---------------- attention ----------------
            # swap PSUM pools: QKV pool's banks get reused for attention
            _psA_cm.__exit__(None, None, None)
            _psB_cm = tc.tile_pool(name="psB", bufs=1, space="PSUM")
            ps = _psB_cm.__enter__()
            def emit_sgroup(pr, qs, g0, gw):
                # one 2-bank psum: cols 0:512 even head, 512:1024 odd head
                pS = ps.tile([P, 1024], F32, tag="S", bufs=2, name="pS")
                for jj in range(gw):
                    js = slice((g0 + jj) * P, (g0 + jj + 1) * P)
                    nc.tensor.matmul(
                        pS[:, jj * P : (jj + 1) * P],
                        KT[0:64, pr, js],
                        QT[0:64, pr, qs],
                        start=True,
                        stop=True,
                        tile_position=(0, 0),
                    )
                    nc.tensor.matmul(
                        pS[:, 512 + jj * P : 512 + (jj + 1) * P],
                        KT[64:128, pr, js],
                        QT[64:128, pr, qs],
                        start=True,
                        stop=True,
                        tile_position=(64, 0),
                    )
                return pS

            def emit_division(h, pO, qs):
                rc = work.tile([P, P], F32, tag="rc", bufs=2, name="rc")
                nc.vector.reciprocal(rc[64:65, :], pO[64:65, :])
                pR = ps.tile([64, P], F32, tag="S", bufs=2, name="pR")
                nc.tensor.matmul(
                    pR[:], ones64[64:65, :], rc[64:65, :], start=True, stop=True
                )
                Rs = work.tile([64, P], F32, tag="Rs", bufs=2, name="Rs")
                nc.scalar.activation(
                    Rs[:], pR[:], mybir.ActivationFunctionType.Copy
                )
                zh = work.tile([64, P], BF, tag="zh", bufs=4, name="zh")
                nc.vector.tensor_tensor(
                    out=zh[:], in0=pO[0:HD, :], in1=Rs[:],
                    op=mybir.AluOpType.mult,
                )
                nc.sync.dma_start(
                    zT[(h % 2) * 64 : (h % 2) * 64 + 64, h // 2, qs], zh[:]
                )

            pending_div = None
            for k_idx in range(NQ):
                L = PAD_L[k_idx]
                qs = slice(k_idx * P, (k_idx + 1) * P)
                for pr in range(H // 2):  # head pair = f-chunk pr
                    h_e, h_o = 2 * pr, 2 * pr + 1
                    pOe = ps.tile([P, P], F32, tag="Oe", bufs=2, name="pOe")
                    pOo = ps.tile([P, P], F32, tag="Oo", bufs=2, name="pOo")
                    n_groups = (L + 3) // 4
                    pending_S = emit_sgroup(pr, qs, 0, min(4, L))
                    # previous pair's softmax division overlaps this pair's
                    # first score group
                    if pending_div is not None:
                        pending_div()
                        pending_div = None
                    for g in range(n_groups):
                        g0 = g * 4
                        gw = min(4, L - g0)
                        pS = pending_S
                        w = gw * P
                        eS = work.tile([P, 1024], BF, tag="eS", bufs=2, name="eS")
                        nc.scalar.activation(
                            eS[:, :].rearrange("p (u q) -> p u q", u=2)[:, :, 0:w],
                            pS[:, :].rearrange("p (u q) -> p u q", u=2)[:, :, 0:w],
                            mybir.ActivationFunctionType.Exp,
                            scale=SCALE,
                        )
                        if g + 1 < n_groups:
                            # next score group issues on PE while ACT runs exp
                            pending_S = emit_sgroup(
                                pr, qs, g0 + 4, min(4, L - g0 - 4)
                            )
                        for jj in range(gw):
                            j = g0 + jj
                            cs = slice(jj * P, (jj + 1) * P)
                            if j >= L - 2:
                                m = m1 if j == L - 2 else m2
                                nc.vector.tensor_tensor(
                                    out=eS[:, :].rearrange(
                                        "p (u q) -> p u q", u=2
                                    )[:, :, cs],
                                    in0=eS[:, :].rearrange(
                                        "p (u q) -> p u q", u=2
                                    )[:, :, cs],
                                    in1=m[:, k_idx : k_idx + 1, :].to_broadcast(
                                        (P, 2, P)
                                    ),
                                    op=mybir.AluOpType.mult,
                                )
                            nc.tensor.matmul(
                                pOe[0 : HD + 1, :],
                                Vx[:, j, h_e, :],
                                eS[:, cs],
                                start=(j == 0),
                                stop=(j == L - 1),
                            )
                            nc.tensor.matmul(
                                pOo[0 : HD + 1, :],
                                Vx[:, j, h_o, :],
                                eS[:, 512 + jj * P : 512 + (jj + 1) * P],
                                start=(j == 0),
                                stop=(j == L - 1),
                            )

                    def _div(h_e=h_e, h_o=h_o, pOe=pOe, pOo=pOo, qs=qs):
                        emit_division(h_e, pOe, qs)
                        emit_division(h_o, pOo, qs)

                    pending_div = _div
            if pending_div is not None:
                pending_div()
                pending_div = None

            # residual: z^T += xTq
            for c in range(EC):
                nc.vector.tensor_tensor(
                    out=zT[:, c, :], in0=zT[:, c, :], in1=xTq[:, c, :],
                    op=mybir.AluOpType.add,
                )

            # ---------------- projection + layernorm ----------------
            _psB_cm.__exit__(None, None, None)
            _psC_cm = tc.tile_pool(name="psC", bufs=1, space="PSUM")
            ps = _psC_cm.__enter__()
            Wp = load_w(WpT_d, "Wp")
            inv_e = 1.0 / float(E)
            for tb in range(NQ):
                y_sb = work.tile([P, E], F32, tag="ysb", bufs=2, name="y_sb")
                for fs in range(E // 512):
                    py = ps.tile([P, 512], F32, tag="mm512", bufs=4, name="py")
                    for c in range(EC):
                        nc.tensor.matmul(
                            py[:],
                            zT[:, c, tb * P : (tb + 1) * P],
                            Wp[:, c, fs * 512 : (fs + 1) * 512],
                            start=(c == 0),
                            stop=(c == EC - 1),
                        )
                    nc.vector.tensor_tensor(
                        out=y_sb[:, fs * 512 : (fs + 1) * 512],
                        in0=py[:],
                        in1=bp_bc[:, fs * 512 : (fs + 1) * 512],
                        op=mybir.AluOpType.add,
                    )
                mean = work.tile([P, 1], F32, tag="stat", bufs=8, name="mean")
                nc.vector.reduce_sum(mean[:], y_sb[:], axis=mybir.AxisListType.X)
                nc.vector.tensor_scalar_mul(mean[:], mean[:], inv_e)
                y_c = work.tile([P, E], F32, tag="yc", bufs=2, name="y_c")
                nc.vector.tensor_scalar(
                    out=y_c[:], in0=y_sb[:], scalar1=mean[:, 0:1], scalar2=None,
                    op0=mybir.AluOpType.subtract,
                )
                var = work.tile([P, 1], F32, tag="stat", bufs=8, name="var")
                nc.scalar.activation(
                    y_sb[:], y_c[:], mybir.ActivationFunctionType.Square,
                    accum_out=var[:],
                )
                rstd = work.tile([P, 1], F32, tag="stat", bufs=8, name="rstd")
                nc.vector.tensor_scalar(
                    out=rstd[:], in0=var[:], scalar1=inv_e, scalar2=float(EPS),
                    op0=mybir.AluOpType.mult, op1=mybir.AluOpType.add,
                )
                nc.scalar.activation(
                    rstd[:], rstd[:], mybir.ActivationFunctionType.Sqrt
                )
                nc.vector.reciprocal(rstd[:], rstd[:])
                nc.vector.tensor_scalar_mul(y_c[:], y_c[:], rstd[:, 0:1])
                nc.vector.tensor_tensor(
                    out=y_c[:], in0=y_c[:], in1=gamma_bc[:],
                    op=mybir.AluOpType.mult,
                )
                nc.vector.tensor_tensor(
                    out=y_c[:], in0=y_c[:], in1=beta_bc[:],
                    op=mybir.AluOpType.add,
                )
                nc.sync.dma_start(y_d[tb, :, :], y_c[:])

            _psC_cm.__exit__(None, None, None)

    _nc_cache["nc"] = nc
    return nc


def _make_masks(blocks):
    m1 = np.zeros((NQ, P, P), np.float32)
    m2 = np.zeros((NQ, P, P), np.float32)
    tril_t = (np.arange(P)[:, None] <= np.arange(P)[None, :]).astype(np.float32)
    for k in range(NQ):
        l_true = blocks[k] + 1
        L = PAD_L[k]
        if l_true == L:
            m1[k] = 1.0
            m2[k] = tril_t
        else:
            assert l_true == L - 1
            m1[k] = tril_t
            m2[k] = 0.0
    # device layout [P(k-local), NQ, P(q-local)]
    return (
        np.ascontiguousarray(m1.transpose(1, 0, 2)).astype(NPBF),
        np.ascontiguousarray(m2.transpose(1, 0, 2)).astype(NPBF),
    )


def kernel(x, Wq, bq, Wk, bk, Wv, bv, Wp, bp, gamma, beta):
    x = np.asarray(x, np.float32)
    nc = _build_nc()

    WqT = np.ascontiguousarray(np.asarray(Wq, np.float32).T).astype(NPBF)
    WkT = np.ascontiguousarray(np.asarray(Wk, np.float32).T).astype(NPBF)
    WvT = np.ascontiguousarray(np.asarray(Wv, np.float32).T).astype(NPBF)
    WpT = np.ascontiguousarray(np.asarray(Wp, np.float32).T).astype(NPBF)
    bqT = np.ascontiguousarray(np.asarray(bq, np.float32).reshape(EC, P).T)
    bkT = np.ascontiguousarray(np.asarray(bk, np.float32).reshape(EC, P).T)
    bv_bc = np.ascontiguousarray(
        np.broadcast_to(np.asarray(bv, np.float32), (P, E))
    )
    bp_bc = np.ascontiguousarray(
        np.broadcast_to(np.asarray(bp, np.float32), (P, E))
    )
    gamma_bc = np.ascontiguousarray(
        np.broadcast_to(np.asarray(gamma, np.float32), (P, E))
    )
    beta_bc = np.ascontiguousarray(
        np.broadcast_to(np.asarray(beta, np.float32), (P, E))
    )
    masks = {0: _make_masks(BLOCKS_A), 1: _make_masks(BLOCKS_B)}

    in_maps = []
    for core in range(8):
        b, h = core // 2, core % 2
        blocks = BLOCKS_A if h == 0 else BLOCKS_B
        own = np.concatenate([np.arange(blk * P, (blk + 1) * P) for blk in blocks])
        xb = x[b]  # (T, E)
        xT = np.ascontiguousarray(xb.T).astype(NPBF)
        xTq = np.ascontiguousarray(xb[own].T).astype(NPBF)
        m1c, m2c = masks[h]
        in_maps.append(
            {
                "xT": xT,
                "xTq": xTq,
                "WqT": WqT,
                "WkT": WkT,
                "WvT": WvT,
                "WpT": WpT,
                "bqT": bqT,
                "bkT": bkT,
                "bv_bc": bv_bc,
                "bp_bc": bp_bc,
                "gamma_bc": gamma_bc,
                "beta_bc": beta_bc,
                "m1": m1c,
                "m2": m2c,
            }
        )

    import os

    trace = bool(int(os.environ.get("MHSA_TRACE", "0")))
    res = run_bass_kernel_spmd(
        nc, in_maps, core_ids=list(range(8)), trace=trace,
        trace_cores=list(range(8)) if trace else None,
    )
    if trace and res.exec_time_ns is not None:
        print(f"HW exec time: {res.exec_time_ns} ns")
        if res.mean_exec_time_ns is not None:
            print(f"HW exec mean across cores: {res.mean_exec_time_ns:.0f} ns")
        kernel.last_exec_time_ns = res.exec_time_ns
        kernel.last_trace = res.instructions_and_trace

    out = np.empty((B, T, E), np.float32)
    for core in range(8):
        b, h = core // 2, core % 2
        blocks = BLOCKS_A if h == 0 else BLOCKS_B
        y = res.results[core]["y"]  # (NQ, P, E)
        for k, blk in enumerate(blocks):
            out[b, blk * P : (blk + 1) * P, :] = y[k]
    return out


# revision 23
# speedup vs baseline: 1.6314x; 1.1494x over previous
"""Multi-head self-attention (B=4, T=2048, E=1024, H=16) on 8 trn2 NeuronCores.

Sharding: core (b, h) = batch b, token-half h. Each core computes K/V for the
full sequence (duplicated within the batch pair), Q for its own 8 query blocks
of 128 tokens, causal attention for those blocks, then the output projection
and LayerNorm for its own tokens. Causal balance: query blocks are paired
(j, 15-j) so both cores process blocks with padded key-lengths 2,4,...,16;
host-supplied mask tiles encode the true causal structure, keeping the
compiled program identical across cores (SPMD).

All matmuls run in bf16 with fp32 PSUM accumulation (validated ~2e-3
scale-relative error vs the fp32 reference).
"""
import json
import numpy as np
import ml_dtypes
from contextlib import ExitStack

import concourse.bass as bass
import concourse.bass_utils as _bass_utils
import concourse.tile as tile
from concourse import mybir
from concourse.bass_utils import run_bass_kernel_spmd

# ----------------------------------------------------------------------------
# Toolchain workarounds for this container's walrus build (see birfix notes):
# 1. EVENT_SEMAPHORE_RANGE_CLEAR InstISA is rejected ("ISA wrong length").
# 2. Engine instructions only carry one semaphore-wait slot; extra waits are
#    peeled onto NoOp carriers on the same engine (order-preserving).
# ----------------------------------------------------------------------------


def _patched_clear_and_free_semaphores(self, sems):
    if not sems:
        return
    sem_nums = [s.num if hasattr(s, "num") else s for s in sems]
    self._state.prepend_free_semaphores(sem_nums)
    for poison_set in self._tile_sem_poison_stack:
        poison_set.update(sem_nums)


def _fix_bir_waits(bir_json: bytes) -> bytes:
    bir = json.loads(bir_json)
    ctr = 0
    changed = False
    for func in bir.get("functions", []):
        for blk in func.get("blocks", []):
            out = []
            for inst in blk.get("instructions", []):
                si = inst.get("sync_info") or {}
                waits = si.get("on_wait") or []
                if len(waits) > 1:
                    for w in waits[:-1]:
                        ctr += 1
                        out.append(
                            {
                                "debug": inst.get("debug"),
                                "engine": inst.get("engine", "SP"),
                                "ins": [],
                                "name": f"IWF-{ctr}",
                                "opcode": "NoOp",
                                "outs": [],
                                "sync_info": {"on_wait": [w]},
                            }
                        )
                    si = dict(si)
                    si["on_wait"] = waits[-1:]
                    inst = dict(inst)
                    inst["sync_info"] = si
                    changed = True
                out.append(inst)
            blk["instructions"] = out
    return json.dumps(bir).encode() if changed else bir_json


_orig_compile_bir_kernel = _bass_utils.compile_bir_kernel


def _patched_compile_bir_kernel(bir_json, tmpdir, neff_name="file.neff"):
    if isinstance(bir_json, str):
        bir_json = bir_json.encode()
    return _orig_compile_bir_kernel(_fix_bir_waits(bir_json), tmpdir, neff_name)


def _install_patches():
    if getattr(bass.Bass, "_mhsa_patched", False):
        return
    bass.Bass.clear_and_free_semaphores = _patched_clear_and_free_semaphores
    bass.Bass._mhsa_patched = True
    _bass_utils.compile_bir_kernel = _patched_compile_bir_kernel
    try:
        import concourse.bass2jax as _b2j

        _b2j.compile_bir_kernel = _patched_compile_bir_kernel
    except ImportError:
        pass


_install_patches()

# ----------------------------------------------------------------------------
# Problem constants (hardcoded per spec)
# ----------------------------------------------------------------------------
B, T, E, H = 4, 2048, 1024, 16
HD = E // H  # 64
P = 128
NB = T // P  # 16 query/key blocks
NQ = 8  # query blocks per core
EC = E // P  # 8 e-chunks
SCALE = 1.0 / float(np.sqrt(T))
EPS = 1e-6
BF = mybir.dt.bfloat16
F32 = mybir.dt.float32
NPBF = ml_dtypes.bfloat16

# query-block assignment: pairs (j, 15-j); core h=0 takes even-j pairs' low
# and high ends so both cores see padded lengths L_k = 2(k+1)
BLOCKS_A = [0, 2, 4, 6, 9, 11, 13, 15]  # true lengths 1,3,5,7,10,12,14,16
BLOCKS_B = [1, 3, 5, 7, 8, 10, 12, 14]  # true lengths 2,4,6,8,9,11,13,15
PAD_L = [2 * (k + 1) for k in range(NQ)]  # 2,4,...,16

_nc_cache = {}


def _build_nc():
    if "nc" in _nc_cache:
        return _nc_cache["nc"]
    nc = bass.Bass(num_devices=8)

    # inputs (per-core)
    xT_d = nc.dram_tensor("xT", [E, T], BF, kind="ExternalInput")
    xTq_d = nc.dram_tensor("xTq", [E, NQ * P], BF, kind="ExternalInput")
    WqT_d = nc.dram_tensor("WqT", [E, E], BF, kind="ExternalInput")
    WkT_d = nc.dram_tensor("WkT", [E, E], BF, kind="ExternalInput")
    WvT_d = nc.dram_tensor("WvT", [E, E], BF, kind="ExternalInput")
    WpT_d = nc.dram_tensor("WpT", [E, E], BF, kind="ExternalInput")
    bqT_d = nc.dram_tensor("bqT", [P, EC], F32, kind="ExternalInput")
    bkT_d = nc.dram_tensor("bkT", [P, EC], F32, kind="ExternalInput")
    bv_d = nc.dram_tensor("bv_bc", [P, E], F32, kind="ExternalInput")
    bp_d = nc.dram_tensor("bp_bc", [P, E], F32, kind="ExternalInput")
    gm_d = nc.dram_tensor("gamma_bc", [P, E], F32, kind="ExternalInput")
    bt_d = nc.dram_tensor("beta_bc", [P, E], F32, kind="ExternalInput")
    m1_d = nc.dram_tensor("m1", [P, NQ, P], BF, kind="ExternalInput")
    m2_d = nc.dram_tensor("m2", [P, NQ, P], BF, kind="ExternalInput")
    y_d = nc.dram_tensor("y", [NQ, P, E], F32, kind="ExternalOutput")

    with tile.TileContext(nc) as tc:
        with ExitStack() as ctx:
            consts = ctx.enter_context(tc.tile_pool(name="consts", bufs=1))
            big = ctx.enter_context(tc.tile_pool(name="big", bufs=1))
            wpool = ctx.enter_context(tc.tile_pool(name="wpool", bufs=1))
            work = ctx.enter_context(tc.tile_pool(name="work", bufs=2))
            # QKV-phase PSUM pool; closed before attention so its banks are
            # reused by the attention pool (8-bank budget)
            _psA_cm = tc.tile_pool(name="psA", bufs=1, space="PSUM")
            ps = _psA_cm.__enter__()

            def load_w(dram, name):
                w = wpool.tile([P, EC, E], BF, tag="w", name=name)
                for c in range(EC):
                    nc.sync.dma_start(
                        w[:, c, :], dram.rearrange("(c p) f -> p c f", p=P)[:, c, :]
                    )
                return w

            # PE-critical loads first: Wk then xT, so the K matmuls can
            # start as soon as possible
            Wk = load_w(WkT_d, "Wk")
            xT = big.tile([P, EC, T], BF)
            for c in range(EC):
                nc.sync.dma_start(
                    xT[:, c, :], xT_d.rearrange("(c p) t -> p c t", p=P)[:, c, :]
                )
            bkT = consts.tile([P, EC], F32)
            nc.sync.dma_start(bkT[:], bkT_d[:, :])
            bv_bc = consts.tile([P, E], F32)
            nc.sync.dma_start(bv_bc[:], bv_d[:, :])
            xTq = big.tile([P, EC, NQ * P], BF)
            for c in range(EC):
                nc.sync.dma_start(
                    xTq[:, c, :], xTq_d.rearrange("(c p) t -> p c t", p=P)[:, c, :]
                )
            bqT = consts.tile([P, EC], F32)
            nc.sync.dma_start(bqT[:], bqT_d[:, :])
            bp_bc = consts.tile([P, E], F32)
            nc.sync.dma_start(bp_bc[:], bp_d[:, :])
            gamma_bc = consts.tile([P, E], F32)
            nc.sync.dma_start(gamma_bc[:], gm_d[:, :])
            beta_bc = consts.tile([P, E], F32)
            nc.sync.dma_start(beta_bc[:], bt_d[:, :])
            m1 = consts.tile([P, NQ, P], BF)
            nc.sync.dma_start(m1[:], m1_d[:, :, :])
            m2 = consts.tile([P, NQ, P], BF)
            nc.sync.dma_start(m2[:], m2_d[:, :, :])
            ones64 = consts.tile([P, 64], F32)
            nc.vector.memset(ones64[:], 1.0)

            # persistent intermediates
            KT = big.tile([P, EC, T], BF)  # K^T  [f, t]
            QT = big.tile([P, EC, NQ * P], BF)  # Q^T  [f, t_own]
            Vx = big.tile([P, NB, H, HD + 1], BF)  # V ext [t, h, d|1]
            zT = big.tile([P, EC, NQ * P], BF)  # z^T  [e, t_own]
            nc.vector.memset(Vx[:, :, :, HD : HD + 1], 1.0)

            # ---------------- K^T = Wk^T.T-chunks x xT + bk ----------------
            for fb in range(EC):
                for ts_ in range(T // 512):
                    pk = ps.tile([P, 512], F32, tag="mm512", bufs=2, name="pk")
                    for c in range(EC):
                        nc.tensor.matmul(
                            pk[:],
                            Wk[:, c, fb * P : (fb + 1) * P],
                            xT[:, c, ts_ * 512 : (ts_ + 1) * 512],
                            start=(c == 0),
                            stop=(c == EC - 1),
                        )
                    nc.vector.tensor_scalar(
                        out=KT[:, fb, ts_ * 512 : (ts_ + 1) * 512],
                        in0=pk[:],
                        scalar1=bkT[:, fb : fb + 1],
                        scalar2=None,
                        op0=mybir.AluOpType.add,
                    )

            # ---------------- V = xT-chunks x Wv^T + bv (t-major, ext) -----
            Wv = load_w(WvT_d, "Wv")
            for tb in range(NB):
                for fs in range(E // 512):
                    pv = ps.tile([P, 512], F32, tag="mm512", bufs=2, name="pv")
                    for c in range(EC):
                        nc.tensor.matmul(
                            pv[:],
                            xT[:, c, tb * P : (tb + 1) * P],
                            Wv[:, c, fs * 512 : (fs + 1) * 512],
                            start=(c == 0),
                            stop=(c == EC - 1),
                        )
                    nc.vector.tensor_tensor(
                        out=Vx[:, tb, fs * 8 : (fs + 1) * 8, 0:HD],
                        in0=pv[:, :].rearrange("p (h d) -> p h d", d=HD),
                        in1=bv_bc[:, fs * 512 : (fs + 1) * 512].rearrange(
                            "p (h d) -> p h d", d=HD
                        ),
                        op=mybir.AluOpType.add,
                    )

            # ---------------- Q^T = Wq^T-chunks x xTq + bq -----------------
            Wq = load_w(WqT_d, "Wq")
            for fb in range(EC):
                for ts_ in range(NQ * P // 512):
                    pq = ps.tile([P, 512], F32, tag="mm512", bufs=2, name="pq")
                    for c in range(EC):
                        nc.tensor.matmul(
                            pq[:],
                            Wq[:, c, fb * P : (fb + 1) * P],
                            xTq[:, c, ts_ * 512 : (ts_ + 1) * 512],
                            start=(c == 0),
                            stop=(c == EC - 1),
                        )
                    nc.vector.tensor_scalar(
                        out=QT[:, fb, ts_ * 512 : (ts_ + 1) * 512],
                        in0=pq[:],
                        scalar1=bqT[:, fb : fb + 1],
                        scalar2=None,
                        op0=mybir.AluOpType.add,
                    )

            # ---------------- attention ----------------
            # swap PSUM pools: QKV pool's banks get reused for attention
            _psA_cm.__exit__(None, None, None)
            _psB_cm = tc.tile_pool(name="psB", bufs=1, space="PSUM")
            ps = _psB_cm.__enter__()
            def emit_sgroup(pr, qs, g0, gw):
                # one 2-bank psum: cols 0:512 even head, 512:1024 odd head
                pS = ps.tile([P, 1024], F32, tag="S", bufs=2, name="pS")
                for jj in range(gw):
                    js = slice((g0 + jj) * P, (g0 + jj + 1) * P)
                    nc.tensor.matmul(
                        pS[:, jj * P : (jj + 1) * P],
                        KT[0:64, pr, js],
                        QT[0:64, pr, qs],
                        start=True,
                        stop=True,
                        tile_position=(0, 0),
                    )
                    nc.tensor.matmul(
                        pS[:, 512 + jj * P : 512 + (jj + 1) * P],
                        KT[64:128, pr, js],
                        QT[64:128, pr, qs],
                        start=True,
                        stop=True,
                        tile_position=(64, 0),
                    )
                return pS

            def emit_division_pair(h_e, pOe, h_o, pOo, qs):
                # per head: copy the sums row to SBUF (DVE), broadcast across
                # 64 partitions with a K=1 matmul, reciprocal, multiply, and
                # scatter into z^T. Both sm copies go first so the PE
                # broadcasts never sit behind other DVE work.
                sms = []
                for pO in (pOe, pOo):
                    sm = work.tile([P, P], F32, tag="sm", bufs=2, name="sm")
                    nc.vector.tensor_copy(sm[64:65, :], pO[64:65, :])
                    sms.append(sm)
                # broadcast into the unused rows 64:128 of the pO bank itself
                for sm, pO in zip(sms, (pOe, pOo)):
                    nc.tensor.matmul(
                        pO[64:128, :], ones64[64:65, :], sm[64:65, :],
                        start=True, stop=True,
                    )
                for h, pO in ((h_e, pOe), (h_o, pOo)):
                    Rs = work.tile([64, P], F32, tag="Rs", bufs=2, name="Rs")
                    nc.vector.reciprocal(Rs[:], pO[64:128, :])
                    zh = work.tile([64, P], BF, tag="zh", bufs=4, name="zh")
                    nc.vector.tensor_tensor(
                        out=zh[:], in0=pO[0:HD, :], in1=Rs[:],
                        op=mybir.AluOpType.mult,
                    )
                    nc.sync.dma_start(
                        zT[(h % 2) * 64 : (h % 2) * 64 + 64, h // 2, qs], zh[:]
                    )

            # flat list of (unit_idx, k_idx, pr, g0, gw); one unit = head pair
            units = []
            flat = []
            for k_idx in range(NQ):
                L = PAD_L[k_idx]
                for pr in range(H // 2):
                    u = len(units)
                    units.append((k_idx, pr, L))
                    for g0 in range(0, L, 4):
                        flat.append((u, g0, min(4, L - g0)))

            pO_cur = None
            pending_div = None
            prev_S = None

            def sgroup_for(idx):
                u, g0, gw = flat[idx]
                k_idx, pr, L = units[u]
                return emit_sgroup(
                    pr, slice(k_idx * P, (k_idx + 1) * P), g0, gw
                )

            prev_S = sgroup_for(0)
            for i, (u, g0, gw) in enumerate(flat):
                k_idx, pr, L = units[u]
                qs = slice(k_idx * P, (k_idx + 1) * P)
                h_e, h_o = 2 * pr, 2 * pr + 1
                if g0 == 0:
                    pO_cur = (
                        ps.tile([P, P], F32, tag="Oe", bufs=2, name="pOe"),
                        ps.tile([P, P], F32, tag="Oo", bufs=2, name="pOo"),
                    )
                pOe, pOo = pO_cur
                pS = prev_S
                w = gw * P
                eS = work.tile([P, 1024], BF, tag="eS", bufs=2, name="eS")
                nc.scalar.activation(
                    eS[:, :].rearrange("p (u q) -> p u q", u=2)[:, :, 0:w],
                    pS[:, :].rearrange("p (u q) -> p u q", u=2)[:, :, 0:w],
                    mybir.ActivationFunctionType.Exp,
                    scale=SCALE,
                )
                if i + 1 < len(flat):
                    # next score group (possibly of the next head pair)
                    # issues on PE while ACT runs this group's exp
                    prev_S = sgroup_for(i + 1)
                if pending_div is not None and g0 == 0:
                    pending_div()
                    pending_div = None
                for jj in range(gw):
                    j = g0 + jj
                    cs = slice(jj * P, (jj + 1) * P)
                    if j >= L - 2:
                        m = m1 if j == L - 2 else m2
                        nc.vector.tensor_tensor(
                            out=eS[:, :].rearrange("p (u q) -> p u q", u=2)[
                                :, :, cs
                            ],
                            in0=eS[:, :].rearrange("p (u q) -> p u q", u=2)[
                                :, :, cs
                            ],
                            in1=m[:, k_idx : k_idx + 1, :].to_broadcast(
                                (P, 2, P)
                            ),
                            op=mybir.AluOpType.mult,
                        )
                    nc.tensor.matmul(
                        pOe[0 : HD + 1, :],
                        Vx[:, j, h_e, :],
                        eS[:, cs],
                        start=(j == 0),
                        stop=(j == L - 1),
                    )
                    nc.tensor.matmul(
                        pOo[0 : HD + 1, :],
                        Vx[:, j, h_o, :],
                        eS[:, 512 + jj * P : 512 + (jj + 1) * P],
                        start=(j == 0),
                        stop=(j == L - 1),
                    )
                if g0 + gw == L:

                    def _div(h_e=h_e, h_o=h_o, pOe=pOe, pOo=pOo, qs=qs):
                        emit_division_pair(h_e, pOe, h_o, pOo, qs)

                    pending_div = _div
            if pending_div is not None:
                pending_div()
                pending_div = None

            # residual: z^T += xTq
            for c in range(EC):
                nc.vector.tensor_tensor(
                    out=zT[:, c, :], in0=zT[:, c, :], in1=xTq[:, c, :],
                    op=mybir.AluOpType.add,
                )

            # ---------------- projection + layernorm ----------------
            _psB_cm.__exit__(None, None, None)
            _psC_cm = tc.tile_pool(name="psC", bufs=1, space="PSUM")
            ps = _psC_cm.__enter__()
            Wp = load_w(WpT_d, "Wp")
            inv_e = 1.0 / float(E)
            for tb in range(NQ):
                y_sb = work.tile([P, E], F32, tag="ysb", bufs=2, name="y_sb")
                for fs in range(E // 512):
                    py = ps.tile([P, 512], F32, tag="mm512", bufs=4, name="py")
                    for c in range(EC):
                        nc.tensor.matmul(
                            py[:],
                            zT[:, c, tb * P : (tb + 1) * P],
                            Wp[:, c, fs * 512 : (fs + 1) * 512],
                            start=(c == 0),
                            stop=(c == EC - 1),
                        )
                    nc.vector.tensor_tensor(
                        out=y_sb[:, fs * 512 : (fs + 1) * 512],
                        in0=py[:],
                        in1=bp_bc[:, fs * 512 : (fs + 1) * 512],
                        op=mybir.AluOpType.add,
                    )
                mean = work.tile([P, 1], F32, tag="stat", bufs=8, name="mean")
                nc.vector.reduce_sum(mean[:], y_sb[:], axis=mybir.AxisListType.X)
                nc.vector.tensor_scalar_mul(mean[:], mean[:], inv_e)
                y_c = work.tile([P, E], F32, tag="yc", bufs=2, name="y_c")
                nc.vector.tensor_scalar(
                    out=y_c[:], in0=y_sb[:], scalar1=mean[:, 0:1], scalar2=None,
                    op0=mybir.AluOpType.subtract,
                )
                var = work.tile([P, 1], F32, tag="stat", bufs=8, name="var")
                nc.scalar.activation(
                    y_sb[:], y_c[:], mybir.ActivationFunctionType.Square,
                    accum_out=var[:],
                )
                rstd = work.tile([P, 1], F32, tag="stat", bufs=8, name="rstd")
                nc.vector.tensor_scalar(
                    out=rstd[:], in0=var[:], scalar1=inv_e, scalar2=float(EPS),
                    op0=mybir.AluOpType.mult, op1=mybir.AluOpType.add,
                )
                nc.scalar.activation(
                    rstd[:], rstd[:], mybir.ActivationFunctionType.Sqrt
                )
                nc.vector.reciprocal(rstd[:], rstd[:])
                nc.vector.tensor_scalar_mul(y_c[:], y_c[:], rstd[:, 0:1])
                nc.vector.tensor_tensor(
                    out=y_c[:], in0=y_c[:], in1=gamma_bc[:],
                    op=mybir.AluOpType.mult,
                )
                nc.vector.tensor_tensor(
                    out=y_c[:], in0=y_c[:], in1=beta_bc[:],
                    op=mybir.AluOpType.add,
                )
                nc.sync.dma_start(y_d[tb, :, :], y_c[:])

            _psC_cm.__exit__(None, None, None)

    _nc_cache["nc"] = nc
    return nc


def _make_masks(blocks):
    m1 = np.zeros((NQ, P, P), np.float32)
    m2 = np.zeros((NQ, P, P), np.float32)
    tril_t = (np.arange(P)[:, None] <= np.arange(P)[None, :]).astype(np.float32)
    for k in range(NQ):
        l_true = blocks[k] + 1
        L = PAD_L[k]
        if l_true == L:
            m1[k] = 1.0
            m2[k] = tril_t
        else:
            assert l_true == L - 1
            m1[k] = tril_t
            m2[k] = 0.0
    # device layout [P(k-local), NQ, P(q-local)]
    return (
        np.ascontiguousarray(m1.transpose(1, 0, 2)).astype(NPBF),
        np.ascontiguousarray(m2.transpose(1, 0, 2)).astype(NPBF),
    )


def kernel(x, Wq, bq, Wk, bk, Wv, bv, Wp, bp, gamma, beta):
    x = np.asarray(x, np.float32)
    nc = _build_nc()

    WqT = np.ascontiguousarray(np.asarray(Wq, np.float32).T).astype(NPBF)
    WkT = np.ascontiguousarray(np.asarray(Wk, np.float32).T).astype(NPBF)
    WvT = np.ascontiguousarray(np.asarray(Wv, np.float32).T).astype(NPBF)
    WpT = np.ascontiguousarray(np.asarray(Wp, np.float32).T).astype(NPBF)
    bqT = np.ascontiguousarray(np.asarray(bq, np.float32).reshape(EC, P).T)
    bkT = np.ascontiguousarray(np.asarray(bk, np.float32).reshape(EC, P).T)
    bv_bc = np.ascontiguousarray(
        np.broadcast_to(np.asarray(bv, np.float32), (P, E))
    )
    bp_bc = np.ascontiguousarray(
        np.broadcast_to(np.asarray(bp, np.float32), (P, E))
    )
    gamma_bc = np.ascontiguousarray(
        np.broadcast_to(np.asarray(gamma, np.float32), (P, E))
    )
    beta_bc = np.ascontiguousarray(
        np.broadcast_to(np.asarray(beta, np.float32), (P, E))
    )
    masks = {0: _make_masks(BLOCKS_A), 1: _make_masks(BLOCKS_B)}

    in_maps = []
    for core in range(8):
        b, h = core // 2, core % 2
        blocks = BLOCKS_A if h == 0 else BLOCKS_B
        own = np.concatenate([np.arange(blk * P, (blk + 1) * P) for blk in blocks])
        xb = x[b]  # (T, E)
        xT = np.ascontiguousarray(xb.T).astype(NPBF)
        xTq = np.ascontiguousarray(xb[own].T).astype(NPBF)
        m1c, m2c = masks[h]
        in_maps.append(
            {
                "xT": xT,
                "xTq": xTq,
                "WqT": WqT,
                "WkT": WkT,
                "WvT": WvT,
                "WpT": WpT,
                "bqT": bqT,
                "bkT": bkT,
                "bv_bc": bv_bc,
                "bp_bc": bp_bc,
                "gamma_bc": gamma_bc,
                "beta_bc": beta_bc,
                "m1": m1c,
                "m2": m2c,
            }
        )

    import os

    trace = bool(int(os.environ.get("MHSA_TRACE", "0")))
    res = run_bass_kernel_spmd(
        nc, in_maps, core_ids=list(range(8)), trace=trace,
        trace_cores=list(range(8)) if trace else None,
    )
    if trace and res.exec_time_ns is not None:
        print(f"HW exec time: {res.exec_time_ns} ns")
        if res.mean_exec_time_ns is not None:
            print(f"HW exec mean across cores: {res.mean_exec_time_ns:.0f} ns")
        kernel.last_exec_time_ns = res.exec_time_ns
        kernel.last_trace = res.instructions_and_trace

    out = np.empty((B, T, E), np.float32)
    for core in range(8):
        b, h = core // 2, core % 2
        blocks = BLOCKS_A if h == 0 else BLOCKS_B
        y = res.results[core]["y"]  # (NQ, P, E)
        for k, blk in enumerate(blocks):
            out[b, blk * P : (blk + 1) * P, :] = y[k]
    return out


# revision 36
# speedup vs baseline: 1.6787x; 1.0290x over previous
"""Multi-head self-attention (B=4, T=2048, E=1024, H=16) on 8 trn2 NeuronCores.

Sharding: core (b, h) = batch b, token-half h. Each core computes K/V for the
full sequence (duplicated within the batch pair), Q for its own 8 query blocks
of 128 tokens, causal attention for those blocks, then the output projection
and LayerNorm for its own tokens. Causal balance: query blocks are paired
(j, 15-j) so both cores process blocks with padded key-lengths 2,4,...,16;
host-supplied mask tiles encode the true causal structure, keeping the
compiled program identical across cores (SPMD).

All matmuls run in bf16 with fp32 PSUM accumulation (validated ~2e-3
scale-relative error vs the fp32 reference).
"""
import json
import numpy as np
import ml_dtypes
from contextlib import ExitStack

import concourse.bass as bass
import concourse.bass_utils as _bass_utils
import concourse.tile as tile
from concourse import mybir
from concourse.bass_utils import run_bass_kernel_spmd

# ----------------------------------------------------------------------------
# Toolchain workarounds for this container's walrus build (see birfix notes):
# 1. EVENT_SEMAPHORE_RANGE_CLEAR InstISA is rejected ("ISA wrong length").
# 2. Engine instructions only carry one semaphore-wait slot; extra waits are
#    peeled onto NoOp carriers on the same engine (order-preserving).
# ----------------------------------------------------------------------------


def _patched_clear_and_free_semaphores(self, sems):
    if not sems:
        return
    sem_nums = [s.num if hasattr(s, "num") else s for s in sems]
    self._state.prepend_free_semaphores(sem_nums)
    for poison_set in self._tile_sem_poison_stack:
        poison_set.update(sem_nums)


def _fix_bir_waits(bir_json: bytes) -> bytes:
    bir = json.loads(bir_json)
    ctr = 0
    changed = False
    for func in bir.get("functions", []):
        for blk in func.get("blocks", []):
            out = []
            for inst in blk.get("instructions", []):
                si = inst.get("sync_info") or {}
                waits = si.get("on_wait") or []
                if len(waits) > 1:
                    for w in waits[:-1]:
                        ctr += 1
                        out.append(
                            {
                                "debug": inst.get("debug"),
                                "engine": inst.get("engine", "SP"),
                                "ins": [],
                                "name": f"IWF-{ctr}",
                                "opcode": "NoOp",
                                "outs": [],
                                "sync_info": {"on_wait": [w]},
                            }
                        )
                    si = dict(si)
                    si["on_wait"] = waits[-1:]
                    inst = dict(inst)
                    inst["sync_info"] = si
                    changed = True
                out.append(inst)
            blk["instructions"] = out
    return json.dumps(bir).encode() if changed else bir_json


_orig_compile_bir_kernel = _bass_utils.compile_bir_kernel


def _patched_compile_bir_kernel(bir_json, tmpdir, neff_name="file.neff"):
    if isinstance(bir_json, str):
        bir_json = bir_json.encode()
    return _orig_compile_bir_kernel(_fix_bir_waits(bir_json), tmpdir, neff_name)


def _install_patches():
    if getattr(bass.Bass, "_mhsa_patched", False):
        return
    bass.Bass.clear_and_free_semaphores = _patched_clear_and_free_semaphores
    bass.Bass._mhsa_patched = True
    _bass_utils.compile_bir_kernel = _patched_compile_bir_kernel
    try:
        import concourse.bass2jax as _b2j

        _b2j.compile_bir_kernel = _patched_compile_bir_kernel
    except ImportError:
        pass


_install_patches()

# ----------------------------------------------------------------------------
# Problem constants (hardcoded per spec)
# ----------------------------------------------------------------------------
B, T, E, H = 4, 2048, 1024, 16
HD = E // H  # 64
P = 128
NB = T // P  # 16 query/key blocks
NQ = 8  # query blocks per core
EC = E // P  # 8 e-chunks
SCALE = 1.0 / float(np.sqrt(T))
EPS = 1e-6
BF = mybir.dt.bfloat16
F32 = mybir.dt.float32
NPBF = ml_dtypes.bfloat16

# query-block assignment: pairs (j, 15-j); core h=0 takes even-j pairs' low
# and high ends so both cores see padded lengths L_k = 2(k+1)
BLOCKS_A = [0, 2, 4, 6, 9, 11, 13, 15]  # true lengths 1,3,5,7,10,12,14,16
BLOCKS_B = [1, 3, 5, 7, 8, 10, 12, 14]  # true lengths 2,4,6,8,9,11,13,15
PAD_L = [2 * (k + 1) for k in range(NQ)]  # 2,4,...,16

_nc_cache = {}


def _build_nc():
    if "nc" in _nc_cache:
        return _nc_cache["nc"]
    nc = bass.Bass(num_devices=8)

    # inputs (per-core)
    xT_d = nc.dram_tensor("xT", [E, T], BF, kind="ExternalInput")
    xTq_d = nc.dram_tensor("xTq", [E, NQ * P], BF, kind="ExternalInput")
    WqT_d = nc.dram_tensor("WqT", [E, E], BF, kind="ExternalInput")
    WkT_d = nc.dram_tensor("WkT", [E, E], BF, kind="ExternalInput")
    WvT_d = nc.dram_tensor("WvT", [E, E], BF, kind="ExternalInput")
    WpT_d = nc.dram_tensor("WpT", [E, E], BF, kind="ExternalInput")
    bqT_d = nc.dram_tensor("bqT", [P, EC], F32, kind="ExternalInput")
    bkT_d = nc.dram_tensor("bkT", [P, EC], F32, kind="ExternalInput")
    bv_d = nc.dram_tensor("bv_bc", [P, E], BF, kind="ExternalInput")
    bp_d = nc.dram_tensor("bp_bc", [P, E], BF, kind="ExternalInput")
    gm_d = nc.dram_tensor("gamma_bc", [P, E], BF, kind="ExternalInput")
    bt_d = nc.dram_tensor("beta_bc", [P, E], BF, kind="ExternalInput")
    m1_d = nc.dram_tensor("m1", [P, NQ, P], BF, kind="ExternalInput")
    m2_d = nc.dram_tensor("m2", [P, NQ, P], BF, kind="ExternalInput")
    y_d = nc.dram_tensor("y", [NQ, P, E], F32, kind="ExternalOutput")

    with tile.TileContext(nc) as tc:
        with ExitStack() as ctx:
            consts = ctx.enter_context(tc.tile_pool(name="consts", bufs=1))
            big = ctx.enter_context(tc.tile_pool(name="big", bufs=1))
            wpool = ctx.enter_context(tc.tile_pool(name="wpool", bufs=1))
            work = ctx.enter_context(tc.tile_pool(name="work", bufs=2))
            # QKV-phase PSUM pool; closed before attention so its banks are
            # reused by the attention pool (8-bank budget)
            _psA_cm = tc.tile_pool(name="psA", bufs=1, space="PSUM")
            ps = _psA_cm.__enter__()

            def load_w(dram, name):
                # two half-tiles in a 3-slot rotation: the next projection's
                # first half streams in while the previous one's second half
                # is still being consumed
                halves = []
                for hf in range(2):
                    w = wpool.tile(
                        [P, EC, E // 2], BF, tag="wh", bufs=3, name=f"{name}{hf}"
                    )
                    for c in range(EC):
                        nc.sync.dma_start(
                            w[:, c, :],
                            dram.rearrange("(c p) f -> p c f", p=P)[
                                :, c, hf * 512 : (hf + 1) * 512
                            ],
                        )
                    halves.append(w)
                return halves

            # PE-critical loads first: Wk then xT, so the K matmuls can
            # start as soon as possible
            Wk = load_w(WkT_d, "Wk")
            xT = big.tile([P, EC, T], BF)
            for c in range(EC):
                nc.sync.dma_start(
                    xT[:, c, :], xT_d.rearrange("(c p) t -> p c t", p=P)[:, c, :]
                )
            bkT = consts.tile([P, EC], F32)
            nc.sync.dma_start(bkT[:], bkT_d[:, :])
            bv_bc = consts.tile([P, E], BF)
            nc.sync.dma_start(bv_bc[:], bv_d[:, :])
            xTq = big.tile([P, EC, NQ * P], BF)
            for c in range(EC):
                nc.sync.dma_start(
                    xTq[:, c, :], xTq_d.rearrange("(c p) t -> p c t", p=P)[:, c, :]
                )
            bqT = consts.tile([P, EC], F32)
            nc.sync.dma_start(bqT[:], bqT_d[:, :])
            bp_bc = consts.tile([P, E], BF)
            nc.sync.dma_start(bp_bc[:], bp_d[:, :])
            gamma_bc = consts.tile([P, E], BF)
            nc.sync.dma_start(gamma_bc[:], gm_d[:, :])
            beta_bc = consts.tile([P, E], BF)
            nc.sync.dma_start(beta_bc[:], bt_d[:, :])
            m1 = consts.tile([P, NQ, P], BF)
            nc.sync.dma_start(m1[:], m1_d[:, :, :])
            m2 = consts.tile([P, NQ, P], BF)
            nc.sync.dma_start(m2[:], m2_d[:, :, :])
            ones64 = consts.tile([P, 64], F32)
            nc.vector.memset(ones64[:], 1.0)

            # persistent intermediates
            KT = big.tile([P, EC, T], BF)  # K^T  [f, t]
            QT = big.tile([P, EC, NQ * P], BF)  # Q^T  [f, t_own]
            Vx = big.tile([P, NB, H, HD + 1], BF)  # V ext [t, h, d|1]
            zT = big.tile([P, EC, NQ * P], BF)  # z^T  [e, t_own]
            nc.vector.memset(Vx[:, :, :, HD : HD + 1], 1.0)

            # ---------------- K^T = Wk^T.T-chunks x xT + bk ----------------
            for fb in range(EC):
                for ts_ in range(T // 512):
                    pk = ps.tile([P, 512], F32, tag="mm512", bufs=4, name="pk")
                    for c in range(EC):
                        nc.tensor.matmul(
                            pk[:],
                            Wk[fb // 4][:, c, (fb % 4) * P : (fb % 4 + 1) * P],
                            xT[:, c, ts_ * 512 : (ts_ + 1) * 512],
                            start=(c == 0),
                            stop=(c == EC - 1),
                        )
                    nc.vector.tensor_scalar(
                        out=KT[:, fb, ts_ * 512 : (ts_ + 1) * 512],
                        in0=pk[:],
                        scalar1=bkT[:, fb : fb + 1],
                        scalar2=None,
                        op0=mybir.AluOpType.add,
                    )

            # ---------------- V = xT-chunks x Wv^T + bv (t-major, ext) -----
            Wv = load_w(WvT_d, "Wv")
            for tb in range(NB):
                for fs in range(E // 512):
                    pv = ps.tile([P, 512], F32, tag="mm512", bufs=4, name="pv")
                    for c in range(EC):
                        nc.tensor.matmul(
                            pv[:],
                            xT[:, c, tb * P : (tb + 1) * P],
                            Wv[fs][:, c, :],
                            start=(c == 0),
                            stop=(c == EC - 1),
                        )
                    nc.vector.tensor_tensor(
                        out=Vx[:, tb, fs * 8 : (fs + 1) * 8, 0:HD],
                        in0=pv[:, :].rearrange("p (h d) -> p h d", d=HD),
                        in1=bv_bc[:, fs * 512 : (fs + 1) * 512].rearrange(
                            "p (h d) -> p h d", d=HD
                        ),
                        op=mybir.AluOpType.add,
                    )

            # ---------------- Q^T = Wq^T-chunks x xTq + bq -----------------
            Wq = load_w(WqT_d, "Wq")
            for fb in range(EC):
                for ts_ in range(NQ * P // 512):
                    pq = ps.tile([P, 512], F32, tag="mm512", bufs=4, name="pq")
                    for c in range(EC):
                        nc.tensor.matmul(
                            pq[:],
                            Wq[fb // 4][:, c, (fb % 4) * P : (fb % 4 + 1) * P],
                            xTq[:, c, ts_ * 512 : (ts_ + 1) * 512],
                            start=(c == 0),
                            stop=(c == EC - 1),
                        )
                    nc.vector.tensor_scalar(
                        out=QT[:, fb, ts_ * 512 : (ts_ + 1) * 512],
                        in0=pq[:],
                        scalar1=bqT[:, fb : fb + 1],
                        scalar2=None,
                        op0=mybir.AluOpType.add,
                    )

            # ---------------- attention ----------------
            # swap PSUM pools: QKV pool's banks get reused for attention
            _psA_cm.__exit__(None, None, None)
            _psB_cm = tc.tile_pool(name="psB", bufs=1, space="PSUM")
            ps = _psB_cm.__enter__()
            def emit_sgroup(pr, qs, g0, gw):
                # one 2-bank psum: cols 0:512 even head, 512:1024 odd head
                pS = ps.tile([P, 1024], F32, tag="S", bufs=2, name="pS")
                for jj in range(gw):
                    js = slice((g0 + jj) * P, (g0 + jj + 1) * P)
                    nc.tensor.matmul(
                        pS[:, jj * P : (jj + 1) * P],
                        KT[0:64, pr, js],
                        QT[0:64, pr, qs],
                        start=True,
                        stop=True,
                        tile_position=(0, 0),
                    )
                    nc.tensor.matmul(
                        pS[:, 512 + jj * P : 512 + (jj + 1) * P],
                        KT[64:128, pr, js],
                        QT[64:128, pr, qs],
                        start=True,
                        stop=True,
                        tile_position=(64, 0),
                    )
                return pS

            def emit_division_pair(h_e, pOe, h_o, pOo, qs):
                # per head: copy the sums row to SBUF (DVE), broadcast across
                # 64 partitions with a K=1 matmul, reciprocal, multiply, and
                # scatter into z^T. Both sm copies go first so the PE
                # broadcasts never sit behind other DVE work.
                sms = []
                for pO in (pOe, pOo):
                    sm = work.tile([P, P], F32, tag="sm", bufs=2, name="sm")
                    nc.vector.tensor_copy(sm[64:65, :], pO[64:65, :])
                    sms.append(sm)
                # broadcast into the unused rows 64:128 of the pO bank itself
                for sm, pO in zip(sms, (pOe, pOo)):
                    nc.tensor.matmul(
                        pO[64:128, :], ones64[64:65, :], sm[64:65, :],
                        start=True, stop=True,
                    )
                for h, pO in ((h_e, pOe), (h_o, pOo)):
                    Rs = work.tile([64, P], F32, tag="Rs", bufs=2, name="Rs")
                    nc.vector.reciprocal(Rs[:], pO[64:128, :])
                    zh = work.tile([64, P], BF, tag="zh", bufs=4, name="zh")
                    nc.vector.tensor_tensor(
                        out=zh[:], in0=pO[0:HD, :], in1=Rs[:],
                        op=mybir.AluOpType.mult,
                    )
                    nc.sync.dma_start(
                        zT[(h % 2) * 64 : (h % 2) * 64 + 64, h // 2, qs], zh[:]
                    )

            # flat list of (unit_idx, k_idx, pr, g0, gw); one unit = head pair
            units = []
            flat = []
            for k_idx in range(NQ):
                L = PAD_L[k_idx]
                for pr in range(H // 2):
                    u = len(units)
                    units.append((k_idx, pr, L))
                    for g0 in range(0, L, 4):
                        flat.append((u, g0, min(4, L - g0)))

            pO_cur = None
            pending_div = None
            prev_S = None

            def sgroup_for(idx):
                u, g0, gw = flat[idx]
                k_idx, pr, L = units[u]
                return emit_sgroup(
                    pr, slice(k_idx * P, (k_idx + 1) * P), g0, gw
                )

            prev_S = sgroup_for(0)
            for i, (u, g0, gw) in enumerate(flat):
                k_idx, pr, L = units[u]
                qs = slice(k_idx * P, (k_idx + 1) * P)
                h_e, h_o = 2 * pr, 2 * pr + 1
                if g0 == 0:
                    pO_cur = (
                        ps.tile([P, P], F32, tag="Oe", bufs=2, name="pOe"),
                        ps.tile([P, P], F32, tag="Oo", bufs=2, name="pOo"),
                    )
                pOe, pOo = pO_cur
                pS = prev_S
                w = gw * P
                eS = work.tile([P, 1024], BF, tag="eS", bufs=3, name="eS")
                nc.scalar.activation(
                    eS[:, :].rearrange("p (u q) -> p u q", u=2)[:, :, 0:w],
                    pS[:, :].rearrange("p (u q) -> p u q", u=2)[:, :, 0:w],
                    mybir.ActivationFunctionType.Exp,
                    scale=SCALE,
                )
                if i + 1 < len(flat):
                    # next score group (possibly of the next head pair)
                    # issues on PE while ACT runs this group's exp
                    prev_S = sgroup_for(i + 1)
                if pending_div is not None and g0 == 0:
                    pending_div()
                    pending_div = None
                for jj in range(gw):
                    j = g0 + jj
                    cs = slice(jj * P, (jj + 1) * P)
                    if j >= L - 2:
                        m = m1 if j == L - 2 else m2
                        nc.vector.tensor_tensor(
                            out=eS[:, :].rearrange("p (u q) -> p u q", u=2)[
                                :, :, cs
                            ],
                            in0=eS[:, :].rearrange("p (u q) -> p u q", u=2)[
                                :, :, cs
                            ],
                            in1=m[:, k_idx : k_idx + 1, :].to_broadcast(
                                (P, 2, P)
                            ),
                            op=mybir.AluOpType.mult,
                        )
                    nc.tensor.matmul(
                        pOe[0 : HD + 1, :],
                        Vx[:, j, h_e, :],
                        eS[:, cs],
                        start=(j == 0),
                        stop=(j == L - 1),
                    )
                    nc.tensor.matmul(
                        pOo[0 : HD + 1, :],
                        Vx[:, j, h_o, :],
                        eS[:, 512 + jj * P : 512 + (jj + 1) * P],
                        start=(j == 0),
                        stop=(j == L - 1),
                    )
                if g0 + gw == L:

                    def _div(h_e=h_e, h_o=h_o, pOe=pOe, pOo=pOo, qs=qs):
                        emit_division_pair(h_e, pOe, h_o, pOo, qs)

                    pending_div = _div
            if pending_div is not None:
                pending_div()
                pending_div = None

            # residual: z^T += xTq
            for c in range(EC):
                nc.vector.tensor_tensor(
                    out=zT[:, c, :], in0=zT[:, c, :], in1=xTq[:, c, :],
                    op=mybir.AluOpType.add,
                )

            # ---------------- projection + layernorm ----------------
            _psB_cm.__exit__(None, None, None)
            _psC_cm = tc.tile_pool(name="psC", bufs=1, space="PSUM")
            ps = _psC_cm.__enter__()
            Wp = load_w(WpT_d, "Wp")
            inv_e = 1.0 / float(E)
            for tb in range(NQ):
                y_sb = work.tile([P, E], F32, tag="ysb", bufs=2, name="y_sb")
                for fs in range(E // 512):
                    py = ps.tile([P, 512], F32, tag="mm512", bufs=4, name="py")
                    for c in range(EC):
                        nc.tensor.matmul(
                            py[:],
                            zT[:, c, tb * P : (tb + 1) * P],
                            Wp[fs][:, c, :],
                            start=(c == 0),
                            stop=(c == EC - 1),
                        )
                    nc.vector.tensor_tensor(
                        out=y_sb[:, fs * 512 : (fs + 1) * 512],
                        in0=py[:],
                        in1=bp_bc[:, fs * 512 : (fs + 1) * 512],
                        op=mybir.AluOpType.add,
                    )
                mean = work.tile([P, 1], F32, tag="stat", bufs=8, name="mean")
                nc.vector.reduce_sum(mean[:], y_sb[:], axis=mybir.AxisListType.X)
                nc.vector.tensor_scalar_mul(mean[:], mean[:], -inv_e)
                y_c = work.tile([P, E], F32, tag="yc", bufs=2, name="y_c")
                nc.scalar.activation(
                    y_c[:], y_sb[:], mybir.ActivationFunctionType.Identity,
                    bias=mean[:, 0:1],
                )
                var = work.tile([P, 1], F32, tag="stat", bufs=8, name="var")
                nc.scalar.activation(
                    y_sb[:], y_c[:], mybir.ActivationFunctionType.Square,
                    accum_out=var[:],
                )
                rstd = work.tile([P, 1], F32, tag="stat", bufs=8, name="rstd")
                nc.vector.tensor_scalar(
                    out=rstd[:], in0=var[:], scalar1=inv_e, scalar2=float(EPS),
                    op0=mybir.AluOpType.mult, op1=mybir.AluOpType.add,
                )
                nc.scalar.activation(
                    rstd[:], rstd[:], mybir.ActivationFunctionType.Sqrt
                )
                nc.vector.reciprocal(rstd[:], rstd[:])
                nc.scalar.activation(
                    y_sb[:], y_c[:], mybir.ActivationFunctionType.Identity,
                    scale=rstd[:, 0:1],
                )
                nc.vector.tensor_tensor(
                    out=y_c[:], in0=y_sb[:], in1=gamma_bc[:],
                    op=mybir.AluOpType.mult,
                )
                nc.vector.tensor_tensor(
                    out=y_c[:], in0=y_c[:], in1=beta_bc[:],
                    op=mybir.AluOpType.add,
                )
                nc.sync.dma_start(y_d[tb, :, :], y_c[:])

            _psC_cm.__exit__(None, None, None)

    _nc_cache["nc"] = nc
    return nc


def _make_masks(blocks):
    m1 = np.zeros((NQ, P, P), np.float32)
    m2 = np.zeros((NQ, P, P), np.float32)
    tril_t = (np.arange(P)[:, None] <= np.arange(P)[None, :]).astype(np.float32)
    for k in range(NQ):
        l_true = blocks[k] + 1
        L = PAD_L[k]
        if l_true == L:
            m1[k] = 1.0
            m2[k] = tril_t
        else:
            assert l_true == L - 1
            m1[k] = tril_t
            m2[k] = 0.0
    # device layout [P(k-local), NQ, P(q-local)]
    return (
        np.ascontiguousarray(m1.transpose(1, 0, 2)).astype(NPBF),
        np.ascontiguousarray(m2.transpose(1, 0, 2)).astype(NPBF),
    )


def kernel(x, Wq, bq, Wk, bk, Wv, bv, Wp, bp, gamma, beta):
    x = np.asarray(x, np.float32)
    nc = _build_nc()

    WqT = np.ascontiguousarray(np.asarray(Wq, np.float32).T).astype(NPBF)
    WkT = np.ascontiguousarray(np.asarray(Wk, np.float32).T).astype(NPBF)
    WvT = np.ascontiguousarray(np.asarray(Wv, np.float32).T).astype(NPBF)
    WpT = np.ascontiguousarray(np.asarray(Wp, np.float32).T).astype(NPBF)
    bqT = np.ascontiguousarray(np.asarray(bq, np.float32).reshape(EC, P).T)
    bkT = np.ascontiguousarray(np.asarray(bk, np.float32).reshape(EC, P).T)
    bv_bc = np.ascontiguousarray(
        np.broadcast_to(np.asarray(bv, np.float32), (P, E))
    ).astype(NPBF)
    bp_bc = np.ascontiguousarray(
        np.broadcast_to(np.asarray(bp, np.float32), (P, E))
    ).astype(NPBF)
    gamma_bc = np.ascontiguousarray(
        np.broadcast_to(np.asarray(gamma, np.float32), (P, E))
    ).astype(NPBF)
    beta_bc = np.ascontiguousarray(
        np.broadcast_to(np.asarray(beta, np.float32), (P, E))
    ).astype(NPBF)
    masks = {0: _make_masks(BLOCKS_A), 1: _make_masks(BLOCKS_B)}

    in_maps = []
    for core in range(8):
        b, h = core // 2, core % 2
        blocks = BLOCKS_A if h == 0 else BLOCKS_B
        own = np.concatenate([np.arange(blk * P, (blk + 1) * P) for blk in blocks])
        xb = x[b]  # (T, E)
        xT = np.ascontiguousarray(xb.T).astype(NPBF)
        xTq = np.ascontiguousarray(xb[own].T).astype(NPBF)
        m1c, m2c = masks[h]
        in_maps.append(
            {
                "xT": xT,
                "xTq": xTq,
                "WqT": WqT,
                "WkT": WkT,
                "WvT": WvT,
                "WpT": WpT,
                "bqT": bqT,
                "bkT": bkT,
                "bv_bc": bv_bc,
                "bp_bc": bp_bc,
                "gamma_bc": gamma_bc,
                "beta_bc": beta_bc,
                "m1": m1c,
                "m2": m2c,
            }
        )

    import os

    trace = bool(int(os.environ.get("MHSA_TRACE", "0")))
    res = run_bass_kernel_spmd(
        nc, in_maps, core_ids=list(range(8)), trace=trace,
        trace_cores=list(range(8)) if trace else None,
    )
    if trace and res.exec_time_ns is not None:
        print(f"HW exec time: {res.exec_time_ns} ns")
        if res.mean_exec_time_ns is not None:
            print(f"HW exec mean across cores: {res.mean_exec_time_ns:.0f} ns")
        kernel.last_exec_time_ns = res.exec_time_ns
        kernel.last_trace = res.instructions_and_trace

    out = np.empty((B, T, E), np.float32)
    for core in range(8):
        b, h = core // 2, core % 2
        blocks = BLOCKS_A if h == 0 else BLOCKS_B
        y = res.results[core]["y"]  # (NQ, P, E)
        for k, blk in enumerate(blocks):
            out[b, blk * P : (blk + 1) * P, :] = y[k]
    return out


# revision 37
# speedup vs baseline: 1.7154x; 1.0219x over previous
"""Multi-head self-attention (B=4, T=2048, E=1024, H=16) on 8 trn2 NeuronCores.

Sharding: core (b, h) = batch b, token-half h. Each core computes K/V for the
full sequence (duplicated within the batch pair), Q for its own 8 query blocks
of 128 tokens, causal attention for those blocks, then the output projection
and LayerNorm for its own tokens. Causal balance: query blocks are paired
(j, 15-j) so both cores process blocks with padded key-lengths 2,4,...,16;
host-supplied mask tiles encode the true causal structure, keeping the
compiled program identical across cores (SPMD).

All matmuls run in bf16 with fp32 PSUM accumulation (validated ~2e-3
scale-relative error vs the fp32 reference).
"""
import json
import numpy as np
import ml_dtypes
from contextlib import ExitStack

import concourse.bass as bass
import concourse.bass_utils as _bass_utils
import concourse.tile as tile
from concourse import mybir
from concourse.bass_utils import run_bass_kernel_spmd

# ----------------------------------------------------------------------------
# Toolchain workarounds for this container's walrus build (see birfix notes):
# 1. EVENT_SEMAPHORE_RANGE_CLEAR InstISA is rejected ("ISA wrong length").
# 2. Engine instructions only carry one semaphore-wait slot; extra waits are
#    peeled onto NoOp carriers on the same engine (order-preserving).
# ----------------------------------------------------------------------------


def _patched_clear_and_free_semaphores(self, sems):
    if not sems:
        return
    sem_nums = [s.num if hasattr(s, "num") else s for s in sems]
    self._state.prepend_free_semaphores(sem_nums)
    for poison_set in self._tile_sem_poison_stack:
        poison_set.update(sem_nums)


def _fix_bir_waits(bir_json: bytes) -> bytes:
    bir = json.loads(bir_json)
    ctr = 0
    changed = False
    for func in bir.get("functions", []):
        for blk in func.get("blocks", []):
            out = []
            for inst in blk.get("instructions", []):
                si = inst.get("sync_info") or {}
                waits = si.get("on_wait") or []
                if len(waits) > 1:
                    for w in waits[:-1]:
                        ctr += 1
                        out.append(
                            {
                                "debug": inst.get("debug"),
                                "engine": inst.get("engine", "SP"),
                                "ins": [],
                                "name": f"IWF-{ctr}",
                                "opcode": "NoOp",
                                "outs": [],
                                "sync_info": {"on_wait": [w]},
                            }
                        )
                    si = dict(si)
                    si["on_wait"] = waits[-1:]
                    inst = dict(inst)
                    inst["sync_info"] = si
                    changed = True
                out.append(inst)
            blk["instructions"] = out
    return json.dumps(bir).encode() if changed else bir_json


_orig_compile_bir_kernel = _bass_utils.compile_bir_kernel


def _patched_compile_bir_kernel(bir_json, tmpdir, neff_name="file.neff"):
    if isinstance(bir_json, str):
        bir_json = bir_json.encode()
    return _orig_compile_bir_kernel(_fix_bir_waits(bir_json), tmpdir, neff_name)


def _install_patches():
    if getattr(bass.Bass, "_mhsa_patched", False):
        return
    bass.Bass.clear_and_free_semaphores = _patched_clear_and_free_semaphores
    bass.Bass._mhsa_patched = True
    _bass_utils.compile_bir_kernel = _patched_compile_bir_kernel
    try:
        import concourse.bass2jax as _b2j

        _b2j.compile_bir_kernel = _patched_compile_bir_kernel
    except ImportError:
        pass


_install_patches()

# ----------------------------------------------------------------------------
# Problem constants (hardcoded per spec)
# ----------------------------------------------------------------------------
B, T, E, H = 4, 2048, 1024, 16
HD = E // H  # 64
P = 128
NB = T // P  # 16 query/key blocks
NQ = 8  # query blocks per core
EC = E // P  # 8 e-chunks
SCALE = 1.0 / float(np.sqrt(T))
EPS = 1e-6
BF = mybir.dt.bfloat16
F32 = mybir.dt.float32
NPBF = ml_dtypes.bfloat16

# query-block assignment: pairs (j, 15-j); core h=0 takes even-j pairs' low
# and high ends so both cores see padded lengths L_k = 2(k+1)
BLOCKS_A = [0, 2, 4, 6, 9, 11, 13, 15]  # true lengths 1,3,5,7,10,12,14,16
BLOCKS_B = [1, 3, 5, 7, 8, 10, 12, 14]  # true lengths 2,4,6,8,9,11,13,15
PAD_L = [2 * (k + 1) for k in range(NQ)]  # 2,4,...,16

_nc_cache = {}


def _build_nc():
    if "nc" in _nc_cache:
        return _nc_cache["nc"]
    nc = bass.Bass(num_devices=8)

    # inputs (per-core)
    xT_d = nc.dram_tensor("xT", [E, T], BF, kind="ExternalInput")
    xTq_d = nc.dram_tensor("xTq", [E, NQ * P], BF, kind="ExternalInput")
    WqT_d = nc.dram_tensor("WqT", [E, E], BF, kind="ExternalInput")
    WkT_d = nc.dram_tensor("WkT", [E, E], BF, kind="ExternalInput")
    WvT_d = nc.dram_tensor("WvT", [E, E], BF, kind="ExternalInput")
    WpT_d = nc.dram_tensor("WpT", [E, E], BF, kind="ExternalInput")
    bqT_d = nc.dram_tensor("bqT", [P, EC], F32, kind="ExternalInput")
    bkT_d = nc.dram_tensor("bkT", [P, EC], F32, kind="ExternalInput")
    bv_d = nc.dram_tensor("bv_bc", [P, E], BF, kind="ExternalInput")
    bp_d = nc.dram_tensor("bp_bc", [P, E], BF, kind="ExternalInput")
    gm_d = nc.dram_tensor("gamma_bc", [P, E], BF, kind="ExternalInput")
    bt_d = nc.dram_tensor("beta_bc", [P, E], BF, kind="ExternalInput")
    m1_d = nc.dram_tensor("m1", [P, NQ, P], BF, kind="ExternalInput")
    m2_d = nc.dram_tensor("m2", [P, NQ, P], BF, kind="ExternalInput")
    y_d = nc.dram_tensor("y", [NQ, P, E], F32, kind="ExternalOutput")

    with tile.TileContext(nc) as tc:
        with ExitStack() as ctx:
            consts = ctx.enter_context(tc.tile_pool(name="consts", bufs=1))
            big = ctx.enter_context(tc.tile_pool(name="big", bufs=1))
            wpool = ctx.enter_context(tc.tile_pool(name="wpool", bufs=1))
            work = ctx.enter_context(tc.tile_pool(name="work", bufs=2))
            # QKV-phase PSUM pool; closed before attention so its banks are
            # reused by the attention pool (8-bank budget)
            _psA_cm = tc.tile_pool(name="psA", bufs=1, space="PSUM")
            ps = _psA_cm.__enter__()

            def load_w(dram, name):
                # two half-tiles in a 3-slot rotation: the next projection's
                # first half streams in while the previous one's second half
                # is still being consumed
                halves = []
                for hf in range(2):
                    w = wpool.tile(
                        [P, EC, E // 2], BF, tag="wh", bufs=3, name=f"{name}{hf}"
                    )
                    for c in range(EC):
                        nc.sync.dma_start(
                            w[:, c, :],
                            dram.rearrange("(c p) f -> p c f", p=P)[
                                :, c, hf * 512 : (hf + 1) * 512
                            ],
                        )
                    halves.append(w)
                return halves

            # PE-critical loads first: Wk then xT, so the K matmuls can
            # start as soon as possible
            Wk = load_w(WkT_d, "Wk")
            xT = big.tile([P, EC, T], BF)
            for c in range(EC):
                nc.sync.dma_start(
                    xT[:, c, :], xT_d.rearrange("(c p) t -> p c t", p=P)[:, c, :]
                )
            bkT = consts.tile([P, EC], F32)
            nc.sync.dma_start(bkT[:], bkT_d[:, :])
            bv_bc = consts.tile([P, E], BF)
            nc.sync.dma_start(bv_bc[:], bv_d[:, :])
            xTq = big.tile([P, EC, NQ * P], BF)
            for c in range(EC):
                nc.sync.dma_start(
                    xTq[:, c, :], xTq_d.rearrange("(c p) t -> p c t", p=P)[:, c, :]
                )
            bqT = consts.tile([P, EC], F32)
            nc.sync.dma_start(bqT[:], bqT_d[:, :])
            bp_bc = consts.tile([P, E], BF)
            nc.sync.dma_start(bp_bc[:], bp_d[:, :])
            gamma_bc = consts.tile([P, E], BF)
            nc.sync.dma_start(gamma_bc[:], gm_d[:, :])
            beta_bc = consts.tile([P, E], BF)
            nc.sync.dma_start(beta_bc[:], bt_d[:, :])
            m1 = consts.tile([P, NQ, P], BF)
            nc.sync.dma_start(m1[:], m1_d[:, :, :])
            m2 = consts.tile([P, NQ, P], BF)
            nc.sync.dma_start(m2[:], m2_d[:, :, :])
            ones64 = consts.tile([P, 64], F32)
            nc.vector.memset(ones64[:], 1.0)

            # persistent intermediates
            KT = big.tile([P, EC, T], BF)  # K^T  [f, t]
            QT = big.tile([P, EC, NQ * P], BF)  # Q^T  [f, t_own]
            Vx = big.tile([P, NB, H, HD + 1], BF)  # V ext [t, h, d|1]
            zT = big.tile([P, EC, NQ * P], BF)  # z^T  [e, t_own]
            nc.vector.memset(Vx[:, :, :, HD : HD + 1], 1.0)

            # ---------------- K^T = Wk^T.T-chunks x xT + bk ----------------
            for fb in range(EC):
                for ts_ in range(T // 512):
                    pk = ps.tile([P, 512], F32, tag="mm512", bufs=4, name="pk")
                    for c in range(EC):
                        nc.tensor.matmul(
                            pk[:],
                            Wk[fb // 4][:, c, (fb % 4) * P : (fb % 4 + 1) * P],
                            xT[:, c, ts_ * 512 : (ts_ + 1) * 512],
                            start=(c == 0),
                            stop=(c == EC - 1),
                        )
                    nc.vector.tensor_scalar(
                        out=KT[:, fb, ts_ * 512 : (ts_ + 1) * 512],
                        in0=pk[:],
                        scalar1=bkT[:, fb : fb + 1],
                        scalar2=None,
                        op0=mybir.AluOpType.add,
                    )

            # ---------------- V = xT-chunks x Wv^T + bv (t-major, ext) -----
            Wv = load_w(WvT_d, "Wv")
            for tb in range(NB):
                for fs in range(E // 512):
                    pv = ps.tile([P, 512], F32, tag="mm512", bufs=4, name="pv")
                    for c in range(EC):
                        nc.tensor.matmul(
                            pv[:],
                            xT[:, c, tb * P : (tb + 1) * P],
                            Wv[fs][:, c, :],
                            start=(c == 0),
                            stop=(c == EC - 1),
                        )
                    nc.vector.tensor_tensor(
                        out=Vx[:, tb, fs * 8 : (fs + 1) * 8, 0:HD],
                        in0=pv[:, :].rearrange("p (h d) -> p h d", d=HD),
                        in1=bv_bc[:, fs * 512 : (fs + 1) * 512].rearrange(
                            "p (h d) -> p h d", d=HD
                        ),
                        op=mybir.AluOpType.add,
                    )

            # ---------------- Q^T = Wq^T-chunks x xTq + bq -----------------
            Wq = load_w(WqT_d, "Wq")
            for fb in range(EC):
                for ts_ in range(NQ * P // 512):
                    pq = ps.tile([P, 512], F32, tag="mm512", bufs=4, name="pq")
                    for c in range(EC):
                        nc.tensor.matmul(
                            pq[:],
                            Wq[fb // 4][:, c, (fb % 4) * P : (fb % 4 + 1) * P],
                            xTq[:, c, ts_ * 512 : (ts_ + 1) * 512],
                            start=(c == 0),
                            stop=(c == EC - 1),
                        )
                    nc.vector.tensor_scalar(
                        out=QT[:, fb, ts_ * 512 : (ts_ + 1) * 512],
                        in0=pq[:],
                        scalar1=bqT[:, fb : fb + 1],
                        scalar2=None,
                        op0=mybir.AluOpType.add,
                    )

            # ---------------- attention ----------------
            # swap PSUM pools: QKV pool's banks get reused for attention
            _psA_cm.__exit__(None, None, None)
            _psB_cm = tc.tile_pool(name="psB", bufs=1, space="PSUM")
            ps = _psB_cm.__enter__()
            def emit_sgroup(pr, qs, g0, gw):
                # one 2-bank psum: cols 0:512 even head, 512:1024 odd head
                pS = ps.tile([P, 1024], F32, tag="S", bufs=3, name="pS")
                for jj in range(gw):
                    js = slice((g0 + jj) * P, (g0 + jj + 1) * P)
                    nc.tensor.matmul(
                        pS[:, jj * P : (jj + 1) * P],
                        KT[0:64, pr, js],
                        QT[0:64, pr, qs],
                        start=True,
                        stop=True,
                        tile_position=(0, 0),
                    )
                    nc.tensor.matmul(
                        pS[:, 512 + jj * P : 512 + (jj + 1) * P],
                        KT[64:128, pr, js],
                        QT[64:128, pr, qs],
                        start=True,
                        stop=True,
                        tile_position=(64, 0),
                    )
                return pS

            def emit_division_pair(h_e, pOe, h_o, pOo, qs):
                # per head: copy the sums row to SBUF (DVE), broadcast across
                # 64 partitions with a K=1 matmul, reciprocal, multiply, and
                # scatter into z^T. Both sm copies go first so the PE
                # broadcasts never sit behind other DVE work.
                sms = []
                for pO in (pOe, pOo):
                    sm = work.tile([P, P], F32, tag="sm", bufs=2, name="sm")
                    nc.vector.tensor_copy(sm[64:65, :], pO[64:65, :])
                    sms.append(sm)
                # broadcast into the unused rows 64:128 of the pO bank itself
                for sm, pO in zip(sms, (pOe, pOo)):
                    nc.tensor.matmul(
                        pO[64:128, :], ones64[64:65, :], sm[64:65, :],
                        start=True, stop=True,
                    )
                for h, pO in ((h_e, pOe), (h_o, pOo)):
                    Rs = work.tile([64, P], F32, tag="Rs", bufs=2, name="Rs")
                    nc.vector.reciprocal(Rs[:], pO[64:128, :])
                    zh = work.tile([64, P], BF, tag="zh", bufs=4, name="zh")
                    nc.vector.tensor_tensor(
                        out=zh[:], in0=pO[0:HD, :], in1=Rs[:],
                        op=mybir.AluOpType.mult,
                    )
                    nc.sync.dma_start(
                        zT[(h % 2) * 64 : (h % 2) * 64 + 64, h // 2, qs], zh[:]
                    )

            # flat list of (unit_idx, k_idx, pr, g0, gw); one unit = head pair
            units = []
            flat = []
            for k_idx in range(NQ):
                L = PAD_L[k_idx]
                for pr in range(H // 2):
                    u = len(units)
                    units.append((k_idx, pr, L))
                    for g0 in range(0, L, 4):
                        flat.append((u, g0, min(4, L - g0)))

            pO_cur = None
            pending_div = None
            prev_S = None

            def sgroup_for(idx):
                u, g0, gw = flat[idx]
                k_idx, pr, L = units[u]
                return emit_sgroup(
                    pr, slice(k_idx * P, (k_idx + 1) * P), g0, gw
                )

            prev_S = sgroup_for(0)
            for i, (u, g0, gw) in enumerate(flat):
                k_idx, pr, L = units[u]
                qs = slice(k_idx * P, (k_idx + 1) * P)
                h_e, h_o = 2 * pr, 2 * pr + 1
                if g0 == 0:
                    pO_cur = (
                        ps.tile([P, P], F32, tag="Oe", bufs=1, name="pOe"),
                        ps.tile([P, P], F32, tag="Oo", bufs=1, name="pOo"),
                    )
                pOe, pOo = pO_cur
                pS = prev_S
                w = gw * P
                eS = work.tile([P, 1024], BF, tag="eS", bufs=3, name="eS")
                nc.scalar.activation(
                    eS[:, :].rearrange("p (u q) -> p u q", u=2)[:, :, 0:w],
                    pS[:, :].rearrange("p (u q) -> p u q", u=2)[:, :, 0:w],
                    mybir.ActivationFunctionType.Exp,
                    scale=SCALE,
                )
                if i + 1 < len(flat):
                    # next score group (possibly of the next head pair)
                    # issues on PE while ACT runs this group's exp
                    prev_S = sgroup_for(i + 1)
                if pending_div is not None and g0 == 0:
                    pending_div()
                    pending_div = None
                for jj in range(gw):
                    j = g0 + jj
                    cs = slice(jj * P, (jj + 1) * P)
                    if j >= L - 2:
                        m = m1 if j == L - 2 else m2
                        nc.vector.tensor_tensor(
                            out=eS[:, :].rearrange("p (u q) -> p u q", u=2)[
                                :, :, cs
                            ],
                            in0=eS[:, :].rearrange("p (u q) -> p u q", u=2)[
                                :, :, cs
                            ],
                            in1=m[:, k_idx : k_idx + 1, :].to_broadcast(
                                (P, 2, P)
                            ),
                            op=mybir.AluOpType.mult,
                        )
                    nc.tensor.matmul(
                        pOe[0 : HD + 1, :],
                        Vx[:, j, h_e, :],
                        eS[:, cs],
                        start=(j == 0),
                        stop=(j == L - 1),
                    )
                    nc.tensor.matmul(
                        pOo[0 : HD + 1, :],
                        Vx[:, j, h_o, :],
                        eS[:, 512 + jj * P : 512 + (jj + 1) * P],
                        start=(j == 0),
                        stop=(j == L - 1),
                    )
                if g0 + gw == L:

                    def _div(h_e=h_e, h_o=h_o, pOe=pOe, pOo=pOo, qs=qs):
                        emit_division_pair(h_e, pOe, h_o, pOo, qs)

                    pending_div = _div
            if pending_div is not None:
                pending_div()
                pending_div = None

            # residual: z^T += xTq
            for c in range(EC):
                nc.vector.tensor_tensor(
                    out=zT[:, c, :], in0=zT[:, c, :], in1=xTq[:, c, :],
                    op=mybir.AluOpType.add,
                )

            # ---------------- projection + layernorm ----------------
            _psB_cm.__exit__(None, None, None)
            _psC_cm = tc.tile_pool(name="psC", bufs=1, space="PSUM")
            ps = _psC_cm.__enter__()
            Wp = load_w(WpT_d, "Wp")
            inv_e = 1.0 / float(E)
            for tb in range(NQ):
                y_sb = work.tile([P, E], F32, tag="ysb", bufs=2, name="y_sb")
                for fs in range(E // 512):
                    py = ps.tile([P, 512], F32, tag="mm512", bufs=4, name="py")
                    for c in range(EC):
                        nc.tensor.matmul(
                            py[:],
                            zT[:, c, tb * P : (tb + 1) * P],
                            Wp[fs][:, c, :],
                            start=(c == 0),
                            stop=(c == EC - 1),
                        )
                    nc.vector.tensor_tensor(
                        out=y_sb[:, fs * 512 : (fs + 1) * 512],
                        in0=py[:],
                        in1=bp_bc[:, fs * 512 : (fs + 1) * 512],
                        op=mybir.AluOpType.add,
                    )
                mean = work.tile([P, 1], F32, tag="stat", bufs=8, name="mean")
                nc.vector.reduce_sum(mean[:], y_sb[:], axis=mybir.AxisListType.X)
                nc.vector.tensor_scalar_mul(mean[:], mean[:], -inv_e)
                y_c = work.tile([P, E], F32, tag="yc", bufs=2, name="y_c")
                nc.scalar.activation(
                    y_c[:], y_sb[:], mybir.ActivationFunctionType.Identity,
                    bias=mean[:, 0:1],
                )
                var = work.tile([P, 1], F32, tag="stat", bufs=8, name="var")
                nc.scalar.activation(
                    y_sb[:], y_c[:], mybir.ActivationFunctionType.Square,
                    accum_out=var[:],
                )
                rstd = work.tile([P, 1], F32, tag="stat", bufs=8, name="rstd")
                nc.vector.tensor_scalar(
                    out=rstd[:], in0=var[:], scalar1=inv_e, scalar2=float(EPS),
                    op0=mybir.AluOpType.mult, op1=mybir.AluOpType.add,
                )
                nc.scalar.activation(
                    rstd[:], rstd[:], mybir.ActivationFunctionType.Sqrt
                )
                nc.vector.reciprocal(rstd[:], rstd[:])
                nc.scalar.activation(
                    y_sb[:], y_c[:], mybir.ActivationFunctionType.Identity,
                    scale=rstd[:, 0:1],
                )
                nc.vector.tensor_tensor(
                    out=y_c[:], in0=y_sb[:], in1=gamma_bc[:],
                    op=mybir.AluOpType.mult,
                )
                nc.vector.tensor_tensor(
                    out=y_c[:], in0=y_c[:], in1=beta_bc[:],
                    op=mybir.AluOpType.add,
                )
                nc.sync.dma_start(y_d[tb, :, :], y_c[:])

            _psC_cm.__exit__(None, None, None)

    _nc_cache["nc"] = nc
    return nc


def _make_masks(blocks):
    m1 = np.zeros((NQ, P, P), np.float32)
    m2 = np.zeros((NQ, P, P), np.float32)
    tril_t = (np.arange(P)[:, None] <= np.arange(P)[None, :]).astype(np.float32)
    for k in range(NQ):
        l_true = blocks[k] + 1
        L = PAD_L[k]
        if l_true == L:
            m1[k] = 1.0
            m2[k] = tril_t
        else:
            assert l_true == L - 1
            m1[k] = tril_t
            m2[k] = 0.0
    # device layout [P(k-local), NQ, P(q-local)]
    return (
        np.ascontiguousarray(m1.transpose(1, 0, 2)).astype(NPBF),
        np.ascontiguousarray(m2.transpose(1, 0, 2)).astype(NPBF),
    )


def kernel(x, Wq, bq, Wk, bk, Wv, bv, Wp, bp, gamma, beta):
    x = np.asarray(x, np.float32)
    nc = _build_nc()

    WqT = np.ascontiguousarray(np.asarray(Wq, np.float32).T).astype(NPBF)
    WkT = np.ascontiguousarray(np.asarray(Wk, np.float32).T).astype(NPBF)
    WvT = np.ascontiguousarray(np.asarray(Wv, np.float32).T).astype(NPBF)
    WpT = np.ascontiguousarray(np.asarray(Wp, np.float32).T).astype(NPBF)
    bqT = np.ascontiguousarray(np.asarray(bq, np.float32).reshape(EC, P).T)
    bkT = np.ascontiguousarray(np.asarray(bk, np.float32).reshape(EC, P).T)
    bv_bc = np.ascontiguousarray(
        np.broadcast_to(np.asarray(bv, np.float32), (P, E))
    ).astype(NPBF)
    bp_bc = np.ascontiguousarray(
        np.broadcast_to(np.asarray(bp, np.float32), (P, E))
    ).astype(NPBF)
    gamma_bc = np.ascontiguousarray(
        np.broadcast_to(np.asarray(gamma, np.float32), (P, E))
    ).astype(NPBF)
    beta_bc = np.ascontiguousarray(
        np.broadcast_to(np.asarray(beta, np.float32), (P, E))
    ).astype(NPBF)
    masks = {0: _make_masks(BLOCKS_A), 1: _make_masks(BLOCKS_B)}

    in_maps = []
    for core in range(8):
        b, h = core // 2, core % 2
        blocks = BLOCKS_A if h == 0 else BLOCKS_B
        own = np.concatenate([np.arange(blk * P, (blk + 1) * P) for blk in blocks])
        xb = x[b]  # (T, E)
        xT = np.ascontiguousarray(xb.T).astype(NPBF)
        xTq = np.ascontiguousarray(xb[own].T).astype(NPBF)
        m1c, m2c = masks[h]
        in_maps.append(
            {
                "xT": xT,
                "xTq": xTq,
                "WqT": WqT,
                "WkT": WkT,
                "WvT": WvT,
                "WpT": WpT,
                "bqT": bqT,
                "bkT": bkT,
                "bv_bc": bv_bc,
                "bp_bc": bp_bc,
                "gamma_bc": gamma_bc,
                "beta_bc": beta_bc,
                "m1": m1c,
                "m2": m2c,
            }
        )

    import os

    trace = bool(int(os.environ.get("MHSA_TRACE", "0")))
    res = run_bass_kernel_spmd(
        nc, in_maps, core_ids=list(range(8)), trace=trace,
        trace_cores=list(range(8)) if trace else None,
    )
    if trace and res.exec_time_ns is not None:
        print(f"HW exec time: {res.exec_time_ns} ns")
        if res.mean_exec_time_ns is not None:
            print(f"HW exec mean across cores: {res.mean_exec_time_ns:.0f} ns")
        kernel.last_exec_time_ns = res.exec_time_ns
        kernel.last_trace = res.instructions_and_trace

    out = np.empty((B, T, E), np.float32)
    for core in range(8):
        b, h = core // 2, core % 2
        blocks = BLOCKS_A if h == 0 else BLOCKS_B
        y = res.results[core]["y"]  # (NQ, P, E)
        for k, blk in enumerate(blocks):
            out[b, blk * P : (blk + 1) * P, :] = y[k]
    return out
